# revision 1
# baseline (speedup 1.0000x reference)
"""Trainium2 Bass kernel for nn_DenseContrastLoss.

Strategy (data-parallel over instances, 8 cores):
  - Host: transpose feats to [N, 784, 256] (pixel-major rows), shard 13
    instances per core, build flat gather indices for the 96 sampled
    pixels per instance (32 anchor + 32 pos + 32 neg).
  - Device (per core): dma_gather the 1248 needed pixel-vectors straight
    from HBM (the rest of feats is never touched), PE-transpose to
    channel-major, run the 2-layer 1x1-conv projection head as matmuls,
    L2-normalize via square/colsum/exp(-0.5 ln), form the 32x32
    anchor-pos and anchor-neg similarity matrices per instance with PE,
    and finish the InfoNCE-style loss with DVE/ACT ops. Outputs 13
    per-instance losses.
  - Host: validity mask from gt_mask areas, masked mean, * LOSS_WEIGHT.
"""

import os
import sys

import numpy as np

if "/opt/trn_rl_repo" not in sys.path:
    sys.path.insert(0, "/opt/trn_rl_repo")

import concourse.bass as bass
import concourse.tile as tile
from concourse import bacc, library_config, mybir
from concourse.bass_utils import run_bass_kernel_spmd

F32 = mybir.dt.float32
F32R = mybir.dt.float32r
I16 = mybir.dt.int16

TAU = 0.07
LOSS_WEIGHT = 1.2
NUM_SAMPLES = 32
C = 256
SIDE = 28
PIX = SIDE * SIDE  # 784
N_INST = 100
N_CORES = 8
NI = 13                      # instances per core (8*13 = 104 >= 100)
SAMP = 3 * NUM_SAMPLES       # 96 sampled pixels per instance
STOT = NI * SAMP             # 1248
NPAD = 1280                  # gather count, multiple of 128
NSLOT = NPAD // 128          # 10
IDXW = NPAD // 16            # 80
CHUNK = 416                  # 1248 = 3 * 416, fits one PSUM bank (fp32)
NCH = STOT // CHUNK          # 3

# float32r shares fp32 storage but streams 4x faster through the PE at
# N>=256 (relaxed-precision multiply mode). Tiles consumed by fp32r
# matmuls must be written as f32r by their producers (BIR verifier rule).
MMDT = F32R if os.environ.get("PROJ_DT", "f32r") == "f32r" else F32


def _build_nc():
    nc = bacc.Bacc("TRN2", target_bir_lowering=False, num_swdge_queues=4)
    featsT = nc.declare_dram_parameter("featsT", [NI * PIX, C], F32, isOutput=False)
    idxw = nc.declare_dram_parameter("idxw", [128, IDXW], I16, isOutput=False)
    w1t = nc.declare_dram_parameter("w1t", [C, C], MMDT, isOutput=False)
    w2t = nc.declare_dram_parameter("w2t", [C, C], MMDT, isOutput=False)
    b1 = nc.declare_dram_parameter("b1", [C], F32, isOutput=False)
    b2 = nc.declare_dram_parameter("b2", [C], F32, isOutput=False)
    identw = nc.declare_dram_parameter("identw", [128, 128], F32, isOutput=False)
    blockr = nc.declare_dram_parameter("blockr", [128, 2], F32, isOutput=False)
    loss = nc.declare_dram_parameter("loss", [14], F32, isOutput=True)

    AT = mybir.ActivationFunctionType
    ALU = mybir.AluOpType
    PSUM = bass.MemorySpace.PSUM

    with tile.TileContext(nc) as tc:
        with tc.tile_pool(name="singles", bufs=1) as singles:
            # Get the GPSIMD library load issued as early as possible: the
            # ~10us Q7 IRAM load gates the gathers, which gate everything.
            nc.gpsimd.load_library(library_config.mlp)
            # Preload the one ACT table set that covers every function this
            # kernel uses (exp, ln, copy, square, relu, identity) so the
            # auto-inserted per-transition loads (1.3us each) never fire.
            nc.scalar.add_instruction(
                mybir.InstLoadActFuncSet(
                    name=nc.get_next_instruction_name(),
                    ins=[],
                    outs=[],
                    act_func_set_id=6,  # natural_log_exp_and_others
                )
            )

            idx_s = singles.tile([128, IDXW], I16)
            nc.sync.dma_start(out=idx_s[:], in_=idxw[:, :])

            W1 = singles.tile([128, 2, C], MMDT)
            nc.sync.dma_start(out=W1[:], in_=w1t.rearrange("(k p) d -> p k d", p=128))
            W2 = singles.tile([128, 2, C], MMDT)
            nc.sync.dma_start(out=W2[:], in_=w2t.rearrange("(k p) d -> p k d", p=128))
            B1 = singles.tile([128, 2], F32)
            nc.sync.dma_start(out=B1[:], in_=b1.rearrange("(m p) -> p m", p=128))
            B2 = singles.tile([128, 2], F32)
            nc.sync.dma_start(out=B2[:], in_=b2.rearrange("(m p) -> p m", p=128))
            ident = singles.tile([128, 128], F32)
            nc.sync.dma_start(out=ident[:], in_=identw[:, :])
            blockt = singles.tile([128, 2], F32)
            nc.sync.dma_start(out=blockt[:], in_=blockr[:, :])

            ones32 = singles.tile([32, 1], F32)
            nc.vector.memset(ones32[:], 1.0)
            onescf = singles.tile([128, 1], F32)
            nc.vector.memset(onescf[:], 1.0)
            onesrf = singles.tile([1, 128], F32)
            nc.vector.memset(onesrf[:], 1.0)
            # memset can't write f32r; round 1.0 through an ACT copy instead
            onesc = singles.tile([128, 1], MMDT)
            nc.scalar.copy(out=onesc[:], in_=onescf[:])
            onesr = singles.tile([1, 128], MMDT)
            nc.scalar.copy(out=onesr[:], in_=onesrf[:])

            with tc.tile_pool(name="big", bufs=1) as big:
                # ---- gather the sampled pixel-vectors from HBM ----
                # The SWDGE descriptor ring holds at most 1024 descriptors
                # per instruction; split across the two SWDGE queues so the
                # two Q7 core-pairs generate descriptors in parallel.
                g = big.tile([128, NSLOT, C], F32)
                base = 0
                for q, cnt in enumerate((384, 384, 384, 128)):
                    s0, s1 = base // 128, (base + cnt) // 128
                    nc.gpsimd.dma_gather(
                        g[:, s0:s1, :], featsT[:, :],
                        idx_s[:, base // 16 : (base + cnt) // 16],
                        cnt, cnt, C, queue_num=q,
                    )
                    base += cnt

                # ---- PE warm-up during the gpsimd library-load window ----
                nwarm = int(os.environ.get("NWARM", "64"))
                if nwarm:
                    with tc.tile_pool(name="warmp", bufs=1, space=PSUM) as warmp:
                        wt = warmp.tile([128, 128], F32, tag="warm")
                        for _ in range(nwarm):
                            nc.tensor.transpose(wt[:], ident[:], ident[:])

                # ---- transpose to channel-major Gt[c, s] ----
                gt = [big.tile([128, NPAD], MMDT, tag=f"gt{h}", name=f"gt{h}")
                      for h in range(2)]
                # transpose groups aligned to gather-queue slot ranges so
                # each group waits on exactly one queue's DMA completion
                with tc.tile_pool(name="tpp", bufs=3, space=PSUM) as tpp:
                    for s0, nsl in ((0, 3), (3, 3), (6, 3), (9, 1)):
                        for h in range(2):
                            tp = tpp.tile([128, 384], F32, tag="tp")
                            for j in range(nsl):
                                nc.tensor.transpose(
                                    tp[:, 128 * j : 128 * (j + 1)],
                                    g[:, s0 + j, 128 * h : 128 * (h + 1)],
                                    ident[:],
                                )
                            nc.vector.tensor_copy(
                                out=gt[h][:, 128 * s0 : 128 * (s0 + nsl)],
                                in_=tp[:, : 128 * nsl],
                            )

                # ---- projection head: P = w2 @ relu(w1 @ G + b1) + b2 ----
                hs = [big.tile([128, STOT], MMDT, tag=f"hs{m}", name=f"hs{m}")
                      for m in range(2)]
                ps = [big.tile([128, STOT], F32, tag=f"ps{m}", name=f"ps{m}")
                      for m in range(2)]
                qs = [big.tile([128, STOT], MMDT, tag=f"qs{m}", name=f"qs{m}")
                      for m in range(2)]
                pn = [big.tile([128, STOT], MMDT, tag=f"pn{m}", name=f"pn{m}")
                      for m in range(2)]

                with tc.tile_pool(name="mmp", bufs=3, space=PSUM) as mmp:
                    for m in range(2):
                        for ch in range(NCH):
                            sl = slice(CHUNK * ch, CHUNK * (ch + 1))
                            hp = mmp.tile([128, CHUNK], F32, tag="hp")
                            for k in range(2):
                                nc.tensor.matmul(
                                    hp[:],
                                    W1[:, k, 128 * m : 128 * (m + 1)],
                                    gt[k][:, sl],
                                    start=(k == 0),
                                    stop=(k == 1),
                                )
                            # relu(x + b1), alternating DVE / ACT
                            if ch % 2 == 0:
                                nc.vector.tensor_scalar(
                                    out=hs[m][:, sl],
                                    in0=hp[:],
                                    scalar1=B1[:, m : m + 1],
                                    scalar2=0.0,
                                    op0=ALU.add,
                                    op1=ALU.max,
                                )
                            else:
                                nc.scalar.activation(
                                    out=hs[m][:, sl], in_=hp[:], func=AT.Relu,
                                    bias=B1[:, m : m + 1],
                                )
                    for m in range(2):
                        for ch in range(NCH):
                            sl = slice(CHUNK * ch, CHUNK * (ch + 1))
                            pp = mmp.tile([128, CHUNK], F32, tag="hp")
                            for k in range(2):
                                nc.tensor.matmul(
                                    pp[:],
                                    W2[:, k, 128 * m : 128 * (m + 1)],
                                    hs[k][:, sl],
                                    start=(k == 0),
                                    stop=(k == 1),
                                )
                            if ch % 2 == 1:
                                nc.vector.tensor_scalar_add(
                                    out=ps[m][:, sl], in0=pp[:],
                                    scalar1=B2[:, m : m + 1],
                                )
                            else:
                                nc.scalar.activation(
                                    out=ps[m][:, sl], in_=pp[:], func=AT.Identity,
                                    bias=B2[:, m : m + 1],
                                )
                            if ch % 2 == 0:
                                nc.vector.tensor_mul(
                                    out=qs[m][:, sl], in0=ps[m][:, sl],
                                    in1=ps[m][:, sl],
                                )
                            else:
                                nc.scalar.square(out=qs[m][:, sl], in_=ps[m][:, sl])

                # ---- per-sample rn = (tau * ||p||^2)^-0.5 row ----
                rrow = big.tile([1, STOT], MMDT)
                with (
                    tc.tile_pool(name="nsqp", bufs=2, space=PSUM) as nsqp,
                    tc.tile_pool(name="rnp", bufs=2, space=PSUM) as rnp,
                    tc.tile_pool(name="simp", bufs=1, space=PSUM) as simp,
                ):
                    for ch in range(NCH):
                        sl = slice(CHUNK * ch, CHUNK * (ch + 1))
                        nsq = nsqp.tile([1, CHUNK], F32, tag="nsq")
                        for m in range(2):
                            nc.tensor.matmul(
                                nsq[:],
                                onesc[:],
                                qs[m][:, sl],
                                start=(m == 0),
                                stop=(m == 1),
                            )
                        # (tau * nsq)^-0.5 = exp(-0.5 * ln(tau * nsq)); Ln and
                        # Exp share one ACT table set (natural_log_exp_and_others)
                        lnt = big.tile([1, CHUNK], F32, tag="lnt", name="lnt", bufs=2)
                        nc.scalar.activation(
                            out=lnt[:], in_=nsq[:], func=AT.Ln, scale=float(TAU)
                        )
                        nc.scalar.activation(
                            out=rrow[:, sl], in_=lnt[:], func=AT.Exp, scale=-0.5
                        )

                    # ---- normalize columns of P ----
                    for ch in range(NCH):
                        sl = slice(CHUNK * ch, CHUNK * (ch + 1))
                        rrep = rnp.tile([128, CHUNK], F32, tag="rrep")
                        nc.tensor.matmul(
                            rrep[:], onesr[:], rrow[:, sl], start=True, stop=True
                        )
                        for m in range(2):
                            nc.vector.tensor_mul(
                                out=pn[m][:, sl], in0=ps[m][:, sl], in1=rrep[:]
                            )

                    # ---- similarity matrices ----
                    # One N=64 matmul per (instance, k-tile) computes
                    # [sim_ap | sim_an] together (pos and neg columns are
                    # adjacent in pn). Two PSUM banks: instances 0-6 / 7-12.
                    GA, GB = 7, NI - 7
                    sa = simp.tile([32, GA * 64], F32, tag="sa")
                    sb = simp.tile([32, GB * 64], F32, tag="sb")
                    for n in range(NI):
                        a0 = SAMP * n
                        dst = sa if n < GA else sb
                        gi = n if n < GA else n - GA
                        osl = slice(64 * gi, 64 * (gi + 1))
                        for k in range(2):
                            nc.tensor.matmul(
                                dst[:, osl],
                                pn[k][:, a0 : a0 + 32],
                                pn[k][:, a0 + 32 : a0 + 96],
                                start=(k == 0),
                                stop=(k == 1),
                            )

                    # ---- softmax-style loss on [32, G, 64] views ----
                    lp = simp.tile([GA, 2], F32, tag="lp")
                    nc.vector.memset(lp[:], 0.0)
                    for half, (st, G) in enumerate(((sa, GA), (sb, GB))):
                        def v4(ap, inner=64, off=0):
                            v = ap.rearrange("p (n m) -> p n m", n=G)
                            return v[:, :, off : off + inner]

                        ap3 = v4(st[:], 32, 0)
                        sm = big.tile([32, G], F32, tag="sm", name="sm", bufs=2)
                        nc.vector.reduce_max(
                            out=sm[:], in_=ap3, axis=mybir.AxisListType.X
                        )
                        smb2 = sm[:].unsqueeze(-1).broadcast_to([32, G, 64])
                        dd = big.tile([32, G * 64], F32, tag="dd", name="dd", bufs=2)
                        nc.vector.tensor_sub(
                            out=dd[:].rearrange("p (n m) -> p n m", n=G),
                            in0=st[:].rearrange("p (n m) -> p n m", n=G),
                            in1=smb2,
                        )
                        ee = big.tile([32, G * 64], F32, tag="ee", name="ee", bufs=2)
                        nc.scalar.activation(out=ee[:], in_=dd[:], func=AT.Exp)

                        ssum = big.tile([32, G], F32, tag="ssum", name="ssum", bufs=2)
                        nc.vector.reduce_sum(
                            out=ssum[:], in_=v4(ee[:], 32, 32),
                            axis=mybir.AxisListType.X,
                        )
                        ssb = ssum[:].unsqueeze(-1).broadcast_to([32, G, 32])

                        tt = big.tile([32, G * 32], F32, tag="tt", name="tt", bufs=2)
                        nc.vector.tensor_add(
                            out=tt[:].rearrange("p (n m) -> p n m", n=G),
                            in0=v4(ee[:], 32, 0), in1=ssb,
                        )
                        lg = big.tile([32, G * 32], F32, tag="lg", name="lg", bufs=2)
                        nc.scalar.activation(out=lg[:], in_=tt[:], func=AT.Ln)
                        ctb = big.tile([32, G * 32], F32, tag="ctb", name="ctb", bufs=2)
                        nc.vector.tensor_sub(
                            out=ctb[:].rearrange("p (n m) -> p n m", n=G),
                            in0=lg[:].rearrange("p (n m) -> p n m", n=G),
                            in1=v4(dd[:], 32, 0),
                        )
                        rowr = big.tile([32, G], F32, tag="rowr", name="rowr", bufs=2)
                        nc.vector.reduce_sum(
                            out=rowr[:],
                            in_=ctb[:].rearrange("p (n m) -> p n m", n=G),
                            axis=mybir.AxisListType.X,
                        )
                        nc.tensor.matmul(
                            lp[:G, half : half + 1], rowr[:], ones32[:],
                            start=True, stop=True,
                        )

                    lout = big.tile([GA, 2], F32)
                    nc.scalar.mul(
                        out=lout[:], in_=lp[:], mul=1.0 / (NUM_SAMPLES * NUM_SAMPLES)
                    )
                    nc.sync.dma_start(
                        out=loss.rearrange("(a b) -> a b", b=2), in_=lout[:]
                    )

    nc.compile()
    return nc


_NC_CACHE = None


def _get_nc():
    global _NC_CACHE
    if _NC_CACHE is None:
        _NC_CACHE = _build_nc()
    return _NC_CACHE


def _host_prep(feats, w1, b1, w2, b2, anchor_inds, pos_inds, neg_inds):
    """Build the 8 per-core input maps."""
    n = feats.shape[0]
    ntot = N_CORES * NI
    # pixel-major feats, padded with copies of instance 0
    ft = np.asarray(feats, dtype=np.float32).reshape(n, C, PIX)
    ft = np.transpose(ft, (0, 2, 1))  # [N, 784, C]
    if ntot > n:
        pad = np.broadcast_to(ft[0], (ntot - n,) + ft.shape[1:])
        ft = np.concatenate([ft, pad], axis=0)
    ft = np.ascontiguousarray(ft.reshape(N_CORES, NI * PIX, C))

    def flat(inds):
        inds = np.asarray(inds)
        f = inds[..., 0].astype(np.int64) * SIDE + inds[..., 1].astype(np.int64)
        if ntot > n:
            f = np.concatenate(
                [f, np.broadcast_to(f[0], (ntot - n,) + f.shape[1:])], axis=0
            )
        return f  # [ntot, 32]

    af, pf, nf = flat(anchor_inds), flat(pos_inds), flat(neg_inds)
    samp = np.concatenate([af, pf, nf], axis=1)  # [ntot, 96]
    base = (np.arange(ntot) % NI)[:, None] * PIX
    rows = (samp + base).reshape(N_CORES, STOT)  # [8, 1248]

    w1t = np.ascontiguousarray(np.asarray(w1, dtype=np.float32).T)
    w2t = np.ascontiguousarray(np.asarray(w2, dtype=np.float32).T)
    b1 = np.ascontiguousarray(np.asarray(b1, dtype=np.float32))
    b2 = np.ascontiguousarray(np.asarray(b2, dtype=np.float32))
    identw = np.eye(128, dtype=np.float32)
    blockr = np.zeros((128, 2), dtype=np.float32)
    for b in range(2):
        blockr[64 * b : 64 * b + 32, b] = 1.0

    in_maps = []
    for c in range(N_CORES):
        idx = np.zeros(NPAD, dtype=np.int16)
        idx[:STOT] = rows[c]
        wrapped = np.ascontiguousarray(idx.reshape(IDXW, 16).T)  # [16, 80]
        idx128 = np.ascontiguousarray(np.tile(wrapped, (8, 1)))  # [128, 80]
        in_maps.append(
            {
                "featsT": ft[c],
                "idxw": idx128,
                "w1t": w1t,
                "w2t": w2t,
                "b1": b1,
                "b2": b2,
                "identw": identw,
                "blockr": blockr,
            }
        )
    return in_maps


def _finalize(loss_per, gt_mask):
    gt = np.asarray(gt_mask)
    area = gt.reshape(gt.shape[0], -1).sum(axis=1)
    valid = (area > NUM_SAMPLES) & (area < PIX - NUM_SAMPLES)
    n_valid = np.float32(valid.sum())
    if n_valid > 0:
        total = np.float32(np.where(valid, loss_per, 0.0).astype(np.float32).sum())
        out = total / max(n_valid, np.float32(1.0))
    else:
        out = np.float32(0.0)
    return np.float32(out * np.float32(LOSS_WEIGHT))


def kernel(feats, w1, b1, w2, b2, gt_mask, anchor_inds, pos_inds, neg_inds,
           _results_hook=None):
    nc = _get_nc()
    in_maps = _host_prep(feats, w1, b1, w2, b2, anchor_inds, pos_inds, neg_inds)
    res = run_bass_kernel_spmd(nc, in_maps, list(range(N_CORES)))
    if _results_hook is not None:
        _results_hook(res)
    parts = []
    for c in range(N_CORES):
        lo = res.results[c]["loss"].reshape(7, 2)
        parts.append(np.concatenate([lo[:, 0], lo[: NI - 7, 1]]))
    loss_per = np.concatenate(parts)[:N_INST]
    return _finalize(loss_per, gt_mask)



# revision 7
# speedup vs baseline: 1.2575x; 1.2575x over previous
"""Trainium2 Bass kernel for nn_DenseContrastLoss.

Strategy (data-parallel over instances, 8 cores x 13 instances):
  - Host: gather the 96 sampled pixel-vectors per instance (32 anchor +
    32 pos + 32 neg; indices are host-known) from feats, transpose to
    channel-major, convert to bf16, and ship [3 chunks x 128 x 2 x 416]
    per core.  Weights ship as bf16 [128, 2, 256].
  - Device (per core): 3-chunk pipeline of
      L1 matmul (bf16) -> relu+b1 -> L2 matmul (bf16) -> +b2
      -> square -> column-sum matmul -> rn = (tau*||p||^2)^-1/2 via
      ln/exp -> rrep broadcast matmul -> pn = p * rn (bf16)
    then 13 anchor x [pos|neg] similarity matmuls (col-tiled into one
    [128, 4*64] PSUM tile: instance n -> partition block n%4, col group
    n//4), and a max-free InfoNCE chain:
      term = ln(1 + sum_m' exp(an) * exp(-ap))
    (the reference's max-subtraction cancels algebraically), finished by
    one [128,4]x[128,4] block-sum matmul -> 16 floats out.
  - Host: validity mask from gt_mask areas, masked mean, * LOSS_WEIGHT.
"""

import sys

import numpy as np

if "/opt/trn_rl_repo" not in sys.path:
    sys.path.insert(0, "/opt/trn_rl_repo")

import ml_dtypes

import concourse.bass as bass
import concourse.tile as tile
from concourse import bacc, mybir
from concourse.bass_utils import run_bass_kernel_spmd

F32 = mybir.dt.float32
F32R = mybir.dt.float32r
BF16 = mybir.dt.bfloat16

TAU = 0.07
LOSS_WEIGHT = 1.2
NUM_SAMPLES = 32
C = 256
SIDE = 28
PIX = SIDE * SIDE  # 784
N_INST = 100
N_CORES = 8
NI = 13                      # instances per core (8*13 = 104 >= 100)
SAMP = 3 * NUM_SAMPLES       # 96 sampled pixels per instance
STOT = NI * SAMP             # 1248
CHUNK = 416                  # 1248 = 3*416; <= 512 (PSUM fp32 bank)
NCH = STOT // CHUNK          # 3


def _build_nc():
    nc = bacc.Bacc("TRN2", target_bir_lowering=False)
    gtc = nc.declare_dram_parameter("gtc", [NCH, 128, 2, CHUNK], BF16,
                                    isOutput=False)
    w1t = nc.declare_dram_parameter("w1t", [128, 2, C], BF16, isOutput=False)
    w2t = nc.declare_dram_parameter("w2t", [128, 2, C], BF16, isOutput=False)
    b1 = nc.declare_dram_parameter("b1", [128, 2], F32, isOutput=False)
    b2 = nc.declare_dram_parameter("b2", [128, 2], F32, isOutput=False)
    blkr = nc.declare_dram_parameter("blkr", [128, 4], F32, isOutput=False)
    loss = nc.declare_dram_parameter("loss", [15], F32, isOutput=True)

    AT = mybir.ActivationFunctionType
    ALU = mybir.AluOpType
    PSUM = bass.MemorySpace.PSUM

    with tile.TileContext(nc) as tc:
        with tc.tile_pool(name="singles", bufs=1) as singles:
            # Preload the one ACT table set covering exp/ln/square/relu/
            # identity/copy so per-transition loads (1.3us) never fire.
            nc.scalar.add_instruction(
                mybir.InstLoadActFuncSet(
                    name=nc.get_next_instruction_name(),
                    ins=[],
                    outs=[],
                    act_func_set_id=6,  # natural_log_exp_and_others
                )
            )

            W1 = singles.tile([128, 2, C], BF16)
            nc.sync.dma_start(out=W1[:], in_=w1t[:, :, :])
            B1 = singles.tile([128, 2], F32)
            nc.sync.dma_start(out=B1[:], in_=b1[:, :])

            gch = [singles.tile([128, 2, CHUNK], BF16, name=f"g{ch}")
                   for ch in range(NCH)]
            nc.sync.dma_start(out=gch[0][:], in_=gtc[0])

            W2 = singles.tile([128, 2, C], BF16)
            nc.sync.dma_start(out=W2[:], in_=w2t[:, :, :])
            B2 = singles.tile([128, 2], F32)
            nc.sync.dma_start(out=B2[:], in_=b2[:, :])
            blk4 = singles.tile([128, 4], F32)
            nc.sync.dma_start(out=blk4[:], in_=blkr[:, :])
            nc.sync.dma_start(out=gch[1][:], in_=gtc[1])
            nc.sync.dma_start(out=gch[2][:], in_=gtc[2])

            onescf = singles.tile([128, 1], F32)
            nc.vector.memset(onescf[:], 1.0)
            onesrf = singles.tile([1, 128], F32)
            nc.vector.memset(onesrf[:], 1.0)
            # memset can't write f32r; round 1.0 through an ACT copy
            onesc = singles.tile([128, 1], F32R)
            nc.scalar.copy(out=onesc[:], in_=onescf[:])
            onesr = singles.tile([1, 128], F32R)
            nc.scalar.copy(out=onesr[:], in_=onesrf[:])

            with tc.tile_pool(name="big", bufs=1) as big:
                hs = [big.tile([128, STOT], BF16, name=f"hs{m}")
                      for m in range(2)]
                ps = [big.tile([128, STOT], BF16, name=f"ps{m}")
                      for m in range(2)]
                pn = [big.tile([128, STOT], BF16, name=f"pn{m}")
                      for m in range(2)]
                rrow = big.tile([1, STOT], F32R)

                with (
                    tc.tile_pool(name="mmp", bufs=4, space=PSUM) as mmp,
                    tc.tile_pool(name="nsqp", bufs=2, space=PSUM) as nsqp,
                    tc.tile_pool(name="simp", bufs=1, space=PSUM) as simp,
                    tc.tile_pool(name="lpp", bufs=1, space=PSUM) as lpp,
                    tc.tile_pool(name="qsp", bufs=4) as qsp,
                ):
                    for ch in range(NCH):
                        sl = slice(CHUNK * ch, CHUNK * (ch + 1))
                        # ---- layer 1 + relu ----
                        for m in range(2):
                            hp = mmp.tile([128, CHUNK], F32, tag="mm")
                            for k in range(2):
                                nc.tensor.matmul(
                                    hp[:],
                                    W1[:, k, 128 * m : 128 * (m + 1)],
                                    gch[ch][:, k, :],
                                    start=(k == 0),
                                    stop=(k == 1),
                                )
                            if m == 0:
                                nc.vector.tensor_scalar(
                                    out=hs[m][:, sl], in0=hp[:],
                                    scalar1=B1[:, m : m + 1], scalar2=0.0,
                                    op0=ALU.add, op1=ALU.max,
                                )
                            else:
                                nc.scalar.activation(
                                    out=hs[m][:, sl], in_=hp[:], func=AT.Relu,
                                    bias=B1[:, m : m + 1],
                                )
                        # ---- layer 2 + bias, square ----
                        qs = []
                        for m in range(2):
                            pp = mmp.tile([128, CHUNK], F32, tag="mm")
                            for k in range(2):
                                nc.tensor.matmul(
                                    pp[:],
                                    W2[:, k, 128 * m : 128 * (m + 1)],
                                    hs[k][:, sl],
                                    start=(k == 0),
                                    stop=(k == 1),
                                )
                            if m == 0:
                                nc.vector.tensor_scalar_add(
                                    out=ps[m][:, sl], in0=pp[:],
                                    scalar1=B2[:, m : m + 1],
                                )
                            else:
                                nc.scalar.activation(
                                    out=ps[m][:, sl], in_=pp[:],
                                    func=AT.Identity, bias=B2[:, m : m + 1],
                                )
                            q = qsp.tile([128, CHUNK], F32R, tag="qs")
                            nc.scalar.activation(
                                out=q[:], in_=pp[:], func=AT.Square,
                                bias=B2[:, m : m + 1],
                            )
                            qs.append(q)
                        # ---- rn = (tau*||p||^2)^-0.5 for this chunk ----
                        nsq = nsqp.tile([1, CHUNK], F32, tag="nsq")
                        for m in range(2):
                            nc.tensor.matmul(
                                nsq[:], onesc[:], qs[m][:],
                                start=(m == 0), stop=(m == 1),
                            )
                        lnt = big.tile([1, CHUNK], F32, tag="lnt", name="lnt",
                                       bufs=2)
                        nc.scalar.activation(
                            out=lnt[:], in_=nsq[:], func=AT.Ln,
                            scale=float(TAU),
                        )
                        nc.scalar.activation(
                            out=rrow[:, sl], in_=lnt[:], func=AT.Exp,
                            scale=-0.5,
                        )
                        # ---- normalize: pn = ps * rn (column broadcast) ----
                        rrep = mmp.tile([128, CHUNK], F32, tag="mm")
                        nc.tensor.matmul(
                            rrep[:], onesr[:], rrow[:, sl], start=True,
                            stop=True,
                        )
                        for m in range(2):
                            nc.vector.tensor_mul(
                                out=pn[m][:, sl], in0=ps[m][:, sl],
                                in1=rrep[:],
                            )

                    # ---- similarities: instance n -> partitions
                    # [32*(n%3), +32), psum cols [64*(n//3), +64) ----
                    # (base partition 96 / quadrant 3 is unusable for
                    # matmul output, so pack 3 blocks x 5 col groups)
                    NJ, NG = 3, 5
                    sim = simp.tile([96, NG * 64], F32, tag="sim")
                    # slots 13, 14 (g=4, j=1,2) are never written by the
                    # matmuls; zero them so the chain reads are defined
                    nc.vector.memset(sim[32:64, 256:320], 0.0)
                    nc.vector.memset(sim[64:96, 256:320], 0.0)
                    for n in range(NI):
                        a0 = SAMP * n
                        j, g = n % NJ, n // NJ
                        dst = sim[32 * j : 32 * (j + 1),
                                  64 * g : 64 * (g + 1)]
                        for k in range(2):
                            nc.tensor.matmul(
                                dst,
                                pn[k][:, a0 : a0 + 32],
                                pn[k][:, a0 + 32 : a0 + 96],
                                start=(k == 0),
                                stop=(k == 1),
                            )
                    sim3 = sim[:].rearrange("p (g m) -> p g m", g=NG)

                    # ---- max-free InfoNCE chain on [96, 5, 64] ----
                    ee = big.tile([96, NG * 32], F32, name="ee")
                    nc.scalar.activation(
                        out=ee[:].rearrange("p (g m) -> p g m", g=NG),
                        in_=sim3[:, :, 32:64], func=AT.Exp,
                    )
                    s4 = big.tile([96, NG], F32, name="s4")
                    nc.vector.reduce_sum(
                        out=s4[:],
                        in_=ee[:].rearrange("p (g m) -> p g m", g=NG),
                        axis=mybir.AxisListType.X,
                    )
                    em = big.tile([96, NG * 32], F32, name="em")
                    nc.scalar.activation(
                        out=em[:].rearrange("p (g m) -> p g m", g=NG),
                        in_=sim3[:, :, 0:32], func=AT.Exp, scale=-1.0,
                    )
                    tt = big.tile([96, NG * 32], F32, name="tt")
                    nc.vector.tensor_mul(
                        out=tt[:].rearrange("p (g m) -> p g m", g=NG),
                        in0=em[:].rearrange("p (g m) -> p g m", g=NG),
                        in1=s4[:].unsqueeze(-1).broadcast_to([96, NG, 32]),
                    )
                    ctb = big.tile([96, NG * 32], F32, name="ctb")
                    nc.scalar.activation(
                        out=ctb[:], in_=tt[:], func=AT.Ln, bias=1.0,
                    )
                    rowr = big.tile([96, NG], F32, name="rowr")
                    nc.vector.reduce_sum(
                        out=rowr[:],
                        in_=ctb[:].rearrange("p (g m) -> p g m", g=NG),
                        axis=mybir.AxisListType.X,
                    )
                    lp = lpp.tile([NG, NJ], F32, tag="lp")
                    nc.tensor.matmul(
                        lp[:], rowr[:], blk4[:96, :NJ], start=True, stop=True,
                    )
                    lout = big.tile([NG, NJ], F32, name="lout")
                    nc.scalar.mul(
                        out=lout[:], in_=lp[:],
                        mul=1.0 / (NUM_SAMPLES * NUM_SAMPLES),
                    )
                    nc.sync.dma_start(
                        out=loss.rearrange("(a b) -> a b", b=NJ), in_=lout[:]
                    )

    nc.compile()
    return nc


_NC_CACHE = None


def _get_nc():
    global _NC_CACHE
    if _NC_CACHE is None:
        _NC_CACHE = _build_nc()
    return _NC_CACHE


def _host_prep(feats, w1, b1, w2, b2, anchor_inds, pos_inds, neg_inds):
    """Build the 8 per-core input maps."""
    n = feats.shape[0]
    ntot = N_CORES * NI
    ff = np.asarray(feats, dtype=np.float32).reshape(n, C, PIX)

    def flat(inds):
        inds = np.asarray(inds)
        f = inds[..., 0].astype(np.int64) * SIDE + inds[..., 1].astype(np.int64)
        if ntot > n:
            f = np.concatenate(
                [f, np.broadcast_to(f[0], (ntot - n,) + f.shape[1:])], axis=0
            )
        return f  # [ntot, 32]

    af, pf, nf = flat(anchor_inds), flat(pos_inds), flat(neg_inds)
    samp = np.concatenate([af, pf, nf], axis=1)  # [ntot, 96]
    # gather: [ntot, C, 96] (pad instances replicate instance 0)
    idx = np.minimum(np.arange(ntot), n - 1)
    g = np.take_along_axis(ff[idx], samp[:, None, :], axis=2)
    # per-core channel-major [C, 1248] -> bf16 chunks [3, 128, 2, 416]
    g = g.reshape(N_CORES, NI, C, SAMP)
    g = np.transpose(g, (0, 2, 1, 3)).reshape(N_CORES, C, STOT)
    g = g.astype(ml_dtypes.bfloat16)
    # [C=2*128, 3*416] -> [3, 128, 2, 416]; C index c = k*128 + p
    g = g.reshape(N_CORES, 2, 128, NCH, CHUNK)
    gtc = np.ascontiguousarray(np.transpose(g, (0, 3, 2, 1, 4)))

    def wprep(w):
        wt = np.asarray(w, dtype=np.float32).T  # [c, d]
        wt = wt.reshape(2, 128, C)              # [k, p, d]
        return np.ascontiguousarray(
            np.transpose(wt, (1, 0, 2)).astype(ml_dtypes.bfloat16)
        )

    w1p, w2p = wprep(w1), wprep(w2)
    b1p = np.ascontiguousarray(
        np.asarray(b1, dtype=np.float32).reshape(2, 128).T
    )
    b2p = np.ascontiguousarray(
        np.asarray(b2, dtype=np.float32).reshape(2, 128).T
    )
    blkr = np.zeros((128, 4), dtype=np.float32)
    for j in range(4):
        blkr[32 * j : 32 * (j + 1), j] = 1.0

    in_maps = []
    for c in range(N_CORES):
        in_maps.append(
            {
                "gtc": gtc[c],
                "w1t": w1p,
                "w2t": w2p,
                "b1": b1p,
                "b2": b2p,
                "blkr": blkr,
            }
        )
    return in_maps


def _finalize(loss_per, gt_mask):
    gt = np.asarray(gt_mask)
    area = gt.reshape(gt.shape[0], -1).sum(axis=1)
    valid = (area > NUM_SAMPLES) & (area < PIX - NUM_SAMPLES)
    n_valid = np.float32(valid.sum())
    if n_valid > 0:
        total = np.float32(np.where(valid, loss_per, 0.0).astype(np.float32).sum())
        out = total / max(n_valid, np.float32(1.0))
    else:
        out = np.float32(0.0)
    return np.float32(out * np.float32(LOSS_WEIGHT))


def kernel(feats, w1, b1, w2, b2, gt_mask, anchor_inds, pos_inds, neg_inds,
           _results_hook=None):
    nc = _get_nc()
    in_maps = _host_prep(feats, w1, b1, w2, b2, anchor_inds, pos_inds, neg_inds)
    res = run_bass_kernel_spmd(nc, in_maps, list(range(N_CORES)))
    if _results_hook is not None:
        _results_hook(res)
    parts = [res.results[c]["loss"][:NI] for c in range(N_CORES)]
    loss_per = np.concatenate(parts)[:N_INST]
    return _finalize(loss_per, gt_mask)


# revision 12
# speedup vs baseline: 1.5655x; 1.2449x over previous
"""Trainium2 Bass kernel for nn_DenseContrastLoss.

Strategy (data-parallel over instances, 8 cores x 13 instances):
  - Host: gather the 96 sampled pixel-vectors per instance (32 anchor +
    32 pos + 32 neg; indices are host-known) from feats, transpose to
    channel-major, convert to bf16, ship [3 chunks x 128 x 2 x 416] per
    core plus bf16 weights.
  - Device (per core), software-pipelined across 3 chunks so the
    in-order PE queue never waits on the DVE/ACT chain:
      L1 (bf16 matmuls, PSUM-bank alternated) -> relu+b1 (DVE)
      L2 (bf16) -> qs = Square(pp+b2) (ACT, f32r)
      colsum matmul -> ln (ACT) -> broadcast matmul -> rn = exp(-ln/2)
      (ACT, [128,chunk] bf16) -> pn = (pp+b2)*rn (DVE
      scalar_tensor_tensor, straight from PSUM)
    then 13 similarity matmuls col-tiled into one [96, 5*64] PSUM tile
    (instance n -> partition block n%3, col group n//3), and a max-free
    InfoNCE chain: term = ln(1 + sum_m' exp(an) * exp(-ap)) (the
    reference's max-subtraction cancels algebraically), finished by a
    [96,5]x[96,3] block-sum matmul -> 15 floats out.
  - Host: validity mask from gt_mask areas, masked mean, * LOSS_WEIGHT.
"""

import sys

import numpy as np

if "/opt/trn_rl_repo" not in sys.path:
    sys.path.insert(0, "/opt/trn_rl_repo")

import ml_dtypes

import concourse.bass as bass
import concourse.tile as tile
from concourse import bacc, mybir
from concourse.bass_utils import run_bass_kernel_spmd

F32 = mybir.dt.float32
F32R = mybir.dt.float32r
BF16 = mybir.dt.bfloat16

TAU = 0.07
LOSS_WEIGHT = 1.2
NUM_SAMPLES = 32
C = 256
SIDE = 28
PIX = SIDE * SIDE  # 784
N_INST = 100
N_CORES = 8
NI = 13                      # instances per core (8*13 = 104 >= 100)
SAMP = 3 * NUM_SAMPLES       # 96 sampled pixels per instance
STOT = NI * SAMP             # 1248
CHUNK = 416                  # 1248 = 3*416; <= 512 (PSUM fp32 bank)
NCH = STOT // CHUNK          # 3
NJ, NG = 3, 5                # sim packing: partition blocks x col groups


def _build_nc():
    nc = bacc.Bacc("TRN2", target_bir_lowering=False)
    gtc = nc.declare_dram_parameter("gtc", [NCH, 128, 2, CHUNK], BF16,
                                    isOutput=False)
    wcm = nc.declare_dram_parameter("wcm", [128, 2, 2, C], BF16,
                                    isOutput=False)
    bcm = nc.declare_dram_parameter("bcm", [128, 8], F32, isOutput=False)
    loss = nc.declare_dram_parameter("loss", [15], F32, isOutput=True)

    AT = mybir.ActivationFunctionType
    ALU = mybir.AluOpType
    PSUM = bass.MemorySpace.PSUM

    with tile.TileContext(nc) as tc:
        with tc.tile_pool(name="singles", bufs=1) as singles:
            # ACT engine: table set first (covers exp/ln/square/relu/copy)
            nc.scalar.add_instruction(
                mybir.InstLoadActFuncSet(
                    name=nc.get_next_instruction_name(),
                    ins=[],
                    outs=[],
                    act_func_set_id=6,  # natural_log_exp_and_others
                )
            )

            # big input DMAs on the sync ring, small ones on the ACT ring
            WC = singles.tile([128, 2, 2, C], BF16)
            nc.sync.dma_start(out=WC[:], in_=wcm[:, :, :, :])
            gch = [singles.tile([128, 2, CHUNK], BF16, name=f"g{ch}")
                   for ch in range(NCH)]
            for ch in range(NCH):
                nc.sync.dma_start(out=gch[ch][:], in_=gtc[ch])
            BC = singles.tile([128, 8], F32)
            nc.scalar.dma_start(out=BC[:], in_=bcm[:, :])
            W1, W2 = WC[:, 0], WC[:, 1]
            B1, B2, blk4 = BC[:, 0:2], BC[:, 2:4], BC[:, 4:8]

            onesrf = singles.tile([1, 128], F32)
            nc.vector.memset(onesrf[:], 1.0)
            onescf = singles.tile([128, 1], F32)
            nc.vector.memset(onescf[:], 1.0)
            onesr = singles.tile([1, 128], F32R)
            nc.scalar.copy(out=onesr[:], in_=onesrf[:])
            onesc = singles.tile([128, 1], F32R)
            nc.scalar.copy(out=onesc[:], in_=onescf[:])

            with tc.tile_pool(name="big", bufs=1) as big:
                hs = [big.tile([128, STOT], BF16, name=f"hs{m}")
                      for m in range(2)]
                pn = [big.tile([128, STOT], BF16, name=f"pn{m}")
                      for m in range(2)]

                with (
                    tc.tile_pool(name="mmp", bufs=5, space=PSUM) as mmp,
                    tc.tile_pool(name="nsqp", bufs=1, space=PSUM) as nsqp,
                    tc.tile_pool(name="lpp", bufs=1, space=PSUM) as lpp,
                    tc.tile_pool(name="simp", bufs=1, space=PSUM) as simp,
                    tc.tile_pool(name="qsp", bufs=4) as qsp,
                ):
                    sim = simp.tile([96, NG * 64], F32, tag="sim")
                    # DVE queue head: zero the two unwritten sim slots
                    # (g=4, j=1,2) so the chain reads are defined
                    nc.vector.memset(sim[32:64, 256:320], 0.0)
                    nc.vector.memset(sim[64:96, 256:320], 0.0)

                    hp = {}
                    pp = {}
                    qs = {}
                    lnt = {}
                    rre = {}

                    def l1(ch):
                        hp[ch] = [mmp.tile([128, CHUNK], F32, tag="mm",
                                           name=f"hp{ch}_{m}")
                                  for m in range(2)]
                        for k in range(2):
                            for m in range(2):
                                nc.tensor.matmul(
                                    hp[ch][m][:],
                                    W1[:, k, 128 * m : 128 * (m + 1)],
                                    gch[ch][:, k, :],
                                    start=(k == 0),
                                    stop=(k == 1),
                                )

                    def relu(ch):
                        sl = slice(CHUNK * ch, CHUNK * (ch + 1))
                        for m in range(2):
                            nc.vector.tensor_scalar(
                                out=hs[m][:, sl], in0=hp[ch][m][:],
                                scalar1=B1[:, m : m + 1], scalar2=0.0,
                                op0=ALU.add, op1=ALU.max,
                            )

                    def l2(ch):
                        sl = slice(CHUNK * ch, CHUNK * (ch + 1))
                        pp[ch] = [mmp.tile([128, CHUNK], F32, tag="mm",
                                           name=f"pp{ch}_{m}")
                                  for m in range(2)]
                        for k in range(2):
                            for m in range(2):
                                nc.tensor.matmul(
                                    pp[ch][m][:],
                                    W2[:, k, 128 * m : 128 * (m + 1)],
                                    hs[k][:, sl],
                                    start=(k == 0),
                                    stop=(k == 1),
                                )

                    def sq(ch):
                        qs[ch] = []
                        for m in range(2):
                            q = qsp.tile([128, CHUNK], F32R, tag="qs")
                            nc.scalar.activation(
                                out=q[:], in_=pp[ch][m][:], func=AT.Square,
                                bias=B2[:, m : m + 1],
                            )
                            qs[ch].append(q)

                    def colsum_ln(ch):
                        # PE: nsq = ones^T (qs0 | qs1); then ACT ln
                        nsq = nsqp.tile([1, CHUNK], F32, tag="nsq")
                        for m in range(2):
                            nc.tensor.matmul(
                                nsq[:], onesc[:], qs[ch][m][:],
                                start=(m == 0), stop=(m == 1),
                            )
                        t = big.tile([1, CHUNK], F32R, tag="lnt",
                                     name="lnt", bufs=2)
                        nc.scalar.activation(
                            out=t[:], in_=nsq[:], func=AT.Ln,
                            scale=float(TAU),
                        )
                        lnt[ch] = t

                    def rrep_mm(ch):
                        # PE: broadcast ln row to 128 partitions
                        r = mmp.tile([128, CHUNK], F32, tag="mm")
                        nc.tensor.matmul(
                            r[:], onesr[:], lnt[ch][:], start=True, stop=True,
                        )
                        rre[ch] = r

                    def rn_exp(ch):
                        # ACT: rn = exp(-0.5*ln(tau*nsq)), bf16 [128,chunk]
                        e = big.tile([128, CHUNK], BF16, tag="rre",
                                     name="rre", bufs=2)
                        nc.scalar.activation(
                            out=e[:], in_=rre[ch][:], func=AT.Exp, scale=-0.5,
                        )
                        rre[ch] = e

                    def pnorm(ch):
                        # DVE: pn = (pp + b2) * rn, straight from PSUM
                        sl = slice(CHUNK * ch, CHUNK * (ch + 1))
                        for m in range(2):
                            nc.vector.scalar_tensor_tensor(
                                out=pn[m][:, sl], in0=pp[ch][m][:],
                                scalar=B2[:, m : m + 1], in1=rre[ch][:],
                                op0=ALU.add, op1=ALU.mult,
                            )

                    def sims(n0, n1):
                        for n in range(n0, n1):
                            a0 = SAMP * n
                            j, g = n % NJ, n // NJ
                            dst = sim[32 * j : 32 * (j + 1),
                                      64 * g : 64 * (g + 1)]
                            for k in range(2):
                                nc.tensor.matmul(
                                    dst,
                                    pn[k][:, a0 : a0 + 32],
                                    pn[k][:, a0 + 32 : a0 + 96],
                                    start=(k == 0),
                                    stop=(k == 1),
                                )

                    # ---- software-pipelined issue order ----
                    l1(0); relu(0); l2(0); sq(0)
                    l1(1); relu(1)
                    colsum_ln(0); rrep_mm(0); rn_exp(0); pnorm(0)
                    l2(1); sq(1)
                    l1(2); relu(2)
                    colsum_ln(1); rrep_mm(1); rn_exp(1); pnorm(1)
                    l2(2); sq(2)
                    sims(0, 4)
                    colsum_ln(2); rrep_mm(2); rn_exp(2); pnorm(2)
                    sims(4, 8)
                    sims(8, NI)

                    # ---- max-free InfoNCE chain on [96, 5, 64] ----
                    sim3 = sim[:].rearrange("p (g m) -> p g m", g=NG)
                    ee = big.tile([96, NG * 32], F32, name="ee")
                    nc.scalar.activation(
                        out=ee[:].rearrange("p (g m) -> p g m", g=NG),
                        in_=sim3[:, :, 32:64], func=AT.Exp,
                    )
                    s4 = big.tile([96, NG], F32, name="s4")
                    nc.vector.reduce_sum(
                        out=s4[:],
                        in_=ee[:].rearrange("p (g m) -> p g m", g=NG),
                        axis=mybir.AxisListType.X,
                    )
                    em = big.tile([96, NG * 32], F32, name="em")
                    nc.scalar.activation(
                        out=em[:].rearrange("p (g m) -> p g m", g=NG),
                        in_=sim3[:, :, 0:32], func=AT.Exp, scale=-1.0,
                    )
                    tt = big.tile([96, NG * 32], F32, name="tt")
                    nc.vector.tensor_mul(
                        out=tt[:].rearrange("p (g m) -> p g m", g=NG),
                        in0=em[:].rearrange("p (g m) -> p g m", g=NG),
                        in1=s4[:].unsqueeze(-1).broadcast_to([96, NG, 32]),
                    )
                    ctb = big.tile([96, NG * 32], F32, name="ctb")
                    nc.scalar.activation(
                        out=ctb[:], in_=tt[:], func=AT.Ln, bias=1.0,
                    )
                    rowr = big.tile([96, NG], F32, name="rowr")
                    nc.vector.reduce_sum(
                        out=rowr[:],
                        in_=ctb[:].rearrange("p (g m) -> p g m", g=NG),
                        axis=mybir.AxisListType.X,
                    )
                    lp = lpp.tile([NG, NJ], F32, tag="lp")
                    nc.tensor.matmul(
                        lp[:], rowr[:], blk4[:96, :NJ], start=True, stop=True,
                    )
                    lout = big.tile([NG, NJ], F32, name="lout")
                    nc.scalar.mul(
                        out=lout[:], in_=lp[:],
                        mul=1.0 / (NUM_SAMPLES * NUM_SAMPLES),
                    )
                    nc.sync.dma_start(
                        out=loss.rearrange("(a b) -> a b", b=NJ), in_=lout[:]
                    )

    nc.compile()
    return nc


_NC_CACHE = None


def _get_nc():
    global _NC_CACHE
    if _NC_CACHE is None:
        _NC_CACHE = _build_nc()
    return _NC_CACHE


def _host_prep(feats, w1, b1, w2, b2, anchor_inds, pos_inds, neg_inds):
    """Build the 8 per-core input maps."""
    n = feats.shape[0]
    ntot = N_CORES * NI
    ff = np.asarray(feats, dtype=np.float32).reshape(n, C, PIX)

    def flat(inds):
        inds = np.asarray(inds)
        f = inds[..., 0].astype(np.int64) * SIDE + inds[..., 1].astype(np.int64)
        if ntot > n:
            f = np.concatenate(
                [f, np.broadcast_to(f[0], (ntot - n,) + f.shape[1:])], axis=0
            )
        return f  # [ntot, 32]

    af, pf, nf = flat(anchor_inds), flat(pos_inds), flat(neg_inds)
    samp = np.concatenate([af, pf, nf], axis=1)  # [ntot, 96]
    idx = np.minimum(np.arange(ntot), n - 1)
    g = np.take_along_axis(ff[idx], samp[:, None, :], axis=2)
    # per-core channel-major [C, 1248] -> bf16 chunks [3, 128, 2, 416]
    g = g.reshape(N_CORES, NI, C, SAMP)
    g = np.transpose(g, (0, 2, 1, 3)).reshape(N_CORES, C, STOT)
    g = g.astype(ml_dtypes.bfloat16)
    g = g.reshape(N_CORES, 2, 128, NCH, CHUNK)  # c = k*128 + p
    gtc = np.ascontiguousarray(np.transpose(g, (0, 3, 2, 1, 4)))

    def wprep(w):
        wt = np.asarray(w, dtype=np.float32).T  # [c, d]
        wt = wt.reshape(2, 128, C)              # [k, p, d]
        return np.transpose(wt, (1, 0, 2)).astype(ml_dtypes.bfloat16)

    wcm = np.ascontiguousarray(
        np.stack([wprep(w1), wprep(w2)], axis=1)
    )  # [128, 2, 2, C]
    bcm = np.zeros((128, 8), dtype=np.float32)
    bcm[:, 0:2] = np.asarray(b1, dtype=np.float32).reshape(2, 128).T
    bcm[:, 2:4] = np.asarray(b2, dtype=np.float32).reshape(2, 128).T
    for j in range(4):
        bcm[32 * j : 32 * (j + 1), 4 + j] = 1.0

    in_maps = []
    for c in range(N_CORES):
        in_maps.append({"gtc": gtc[c], "wcm": wcm, "bcm": bcm})
    return in_maps


def _finalize(loss_per, gt_mask):
    gt = np.asarray(gt_mask)
    area = gt.reshape(gt.shape[0], -1).sum(axis=1)
    valid = (area > NUM_SAMPLES) & (area < PIX - NUM_SAMPLES)
    n_valid = np.float32(valid.sum())
    if n_valid > 0:
        total = np.float32(np.where(valid, loss_per, 0.0).astype(np.float32).sum())
        out = total / max(n_valid, np.float32(1.0))
    else:
        out = np.float32(0.0)
    return np.float32(out * np.float32(LOSS_WEIGHT))


def kernel(feats, w1, b1, w2, b2, gt_mask, anchor_inds, pos_inds, neg_inds,
           _results_hook=None):
    nc = _get_nc()
    in_maps = _host_prep(feats, w1, b1, w2, b2, anchor_inds, pos_inds, neg_inds)
    res = run_bass_kernel_spmd(nc, in_maps, list(range(N_CORES)))
    if _results_hook is not None:
        _results_hook(res)
    parts = [res.results[c]["loss"][:NI] for c in range(N_CORES)]
    loss_per = np.concatenate(parts)[:N_INST]
    return _finalize(loss_per, gt_mask)


# revision 18
# speedup vs baseline: 1.5717x; 1.0040x over previous
"""Trainium2 Bass kernel for nn_DenseContrastLoss.

Strategy (data-parallel over instances, 8 cores x 13 instances):
  - Host: gather the 96 sampled pixel-vectors per instance (32 anchor +
    32 pos + 32 neg; indices are host-known) from feats, transpose to
    channel-major, convert to bf16, ship [3 chunks x 128 x 2 x 416] per
    core plus bf16 weights.
  - Device (per core), software-pipelined across 3 chunks so the
    in-order PE queue never waits on the DVE/ACT chain:
      L1 (bf16 matmuls, PSUM-bank alternated) -> relu+b1 (DVE)
      L2 (bf16) -> qs = Square(pp+b2) (ACT, f32r)
      colsum matmul -> ln (ACT) -> broadcast matmul -> rn = exp(-ln/2)
      (ACT, [128,chunk] bf16) -> pn = (pp+b2)*rn (DVE
      scalar_tensor_tensor, straight from PSUM)
    then 13 similarity matmuls col-tiled into one [96, 5*64] PSUM tile
    (instance n -> partition block n%3, col group n//3), and a max-free
    InfoNCE chain: term = ln(1 + sum_m' exp(an) * exp(-ap)) (the
    reference's max-subtraction cancels algebraically), finished by a
    [96,5]x[96,3] block-sum matmul -> 15 floats out.
  - Host: validity mask from gt_mask areas, masked mean, * LOSS_WEIGHT.
"""

import sys

import numpy as np

if "/opt/trn_rl_repo" not in sys.path:
    sys.path.insert(0, "/opt/trn_rl_repo")

import ml_dtypes

import concourse.bass as bass
import concourse.tile as tile
from concourse import bacc, mybir
from concourse.bass_utils import run_bass_kernel_spmd

F32 = mybir.dt.float32
F32R = mybir.dt.float32r
BF16 = mybir.dt.bfloat16

TAU = 0.07
LOSS_WEIGHT = 1.2
NUM_SAMPLES = 32
C = 256
SIDE = 28
PIX = SIDE * SIDE  # 784
N_INST = 100
N_CORES = 8
NI = 13                      # instances per core (8*13 = 104 >= 100)
SAMP = 3 * NUM_SAMPLES       # 96 sampled pixels per instance
STOT = NI * SAMP             # 1248
# uneven chunks (each <= 512, the PSUM fp32 bank limit): a small last
# chunk shortens the serial norm->sim->loss tail
CHUNKS = [512, 448, 288]
COFF = [0, 512, 960]
NCH = len(CHUNKS)
# instances fully covered once chunk ch's pn is written
SIMS_AT = [(0, 5), (5, 10), (10, 13)]
NJ, NG = 3, 5                # sim packing: partition blocks x col groups
NWARM = 16                   # PE warm-up matmuls during the input DMA


def _build_nc():
    nc = bacc.Bacc("TRN2", target_bir_lowering=False)
    gts = [nc.declare_dram_parameter(f"gt{ch}", [128, 2, CHUNKS[ch]], BF16,
                                     isOutput=False)
           for ch in range(NCH)]
    wcm = nc.declare_dram_parameter("wcm", [128, 2, 2, C], BF16,
                                    isOutput=False)
    bcm = nc.declare_dram_parameter("bcm", [128, 8], F32, isOutput=False)
    loss = nc.declare_dram_parameter("loss", [15], F32, isOutput=True)

    AT = mybir.ActivationFunctionType
    ALU = mybir.AluOpType
    PSUM = bass.MemorySpace.PSUM

    with tile.TileContext(nc) as tc:
        with tc.tile_pool(name="singles", bufs=1) as singles:
            # ACT engine: table set first (covers exp/ln/square/relu/copy)
            nc.scalar.add_instruction(
                mybir.InstLoadActFuncSet(
                    name=nc.get_next_instruction_name(),
                    ins=[],
                    outs=[],
                    act_func_set_id=6,  # natural_log_exp_and_others
                )
            )

            # big input DMAs on the sync ring, small ones on the ACT ring
            WC = singles.tile([128, 2, 2, C], BF16)
            nc.sync.dma_start(out=WC[:], in_=wcm[:, :, :, :])
            gch = [singles.tile([128, 2, CHUNKS[ch]], BF16, name=f"g{ch}")
                   for ch in range(NCH)]
            for ch in range(NCH):
                nc.sync.dma_start(out=gch[ch][:], in_=gts[ch][:, :, :])
            BC = singles.tile([128, 8], F32)
            nc.scalar.dma_start(out=BC[:], in_=bcm[:, :])
            W1, W2 = WC[:, 0], WC[:, 1]
            B1, B2, blk4 = BC[:, 0:2], BC[:, 2:4], BC[:, 4:8]

            onesrf = singles.tile([1, 128], F32)
            nc.vector.memset(onesrf[:], 1.0)
            onescf = singles.tile([128, 1], F32)
            nc.vector.memset(onescf[:], 1.0)
            onesr = singles.tile([1, 128], F32R)
            nc.scalar.copy(out=onesr[:], in_=onesrf[:])
            onesc = singles.tile([128, 1], F32R)
            nc.scalar.copy(out=onesc[:], in_=onescf[:])

            with tc.tile_pool(name="big", bufs=1) as big:
                hs = [big.tile([128, STOT], BF16, name=f"hs{m}")
                      for m in range(2)]
                pn = [big.tile([128, STOT], BF16, name=f"pn{m}")
                      for m in range(2)]

                with (
                    tc.tile_pool(name="mmp", bufs=5, space=PSUM) as mmp,
                    tc.tile_pool(name="nsqp", bufs=1, space=PSUM) as nsqp,
                    tc.tile_pool(name="lpp", bufs=1, space=PSUM) as lpp,
                    tc.tile_pool(name="simp", bufs=1, space=PSUM) as simp,
                    tc.tile_pool(name="qsp", bufs=4) as qsp,
                ):
                    sim = simp.tile([96, NG * 64], F32, tag="sim")

                    # PE warm-up during the input-DMA window: ramps HAM to
                    # K=8/8 before the real matmuls arrive.  Writes land in
                    # the sim bank and are overwritten later (start=True).
                    warm = singles.tile([128, 416], BF16, name="warm")
                    nc.vector.memset(warm[:], 1.0)
                    for _ in range(NWARM):
                        nc.tensor.matmul(
                            sim[:96, :], warm[:, :96], warm[:, :320],
                            start=True, stop=True,
                        )

                    hp = {}
                    pp = {}
                    qs = {}
                    lnt = {}
                    rre = {}

                    def l1(ch):
                        cw = CHUNKS[ch]
                        hp[ch] = [mmp.tile([128, 512], F32, tag="mm",
                                           name=f"hp{ch}_{m}")
                                  for m in range(2)]
                        for k in range(2):
                            for m in range(2):
                                nc.tensor.matmul(
                                    hp[ch][m][:, :cw],
                                    W1[:, k, 128 * m : 128 * (m + 1)],
                                    gch[ch][:, k, :],
                                    start=(k == 0),
                                    stop=(k == 1),
                                )

                    def relu(ch):
                        sl = slice(COFF[ch], COFF[ch] + CHUNKS[ch])
                        for m in range(2):
                            nc.vector.tensor_scalar(
                                out=hs[m][:, sl], in0=hp[ch][m][:, :CHUNKS[ch]],
                                scalar1=B1[:, m : m + 1], scalar2=0.0,
                                op0=ALU.add, op1=ALU.max,
                            )

                    def l2(ch):
                        sl = slice(COFF[ch], COFF[ch] + CHUNKS[ch])
                        pp[ch] = [mmp.tile([128, 512], F32, tag="mm",
                                           name=f"pp{ch}_{m}")
                                  for m in range(2)]
                        for k in range(2):
                            for m in range(2):
                                nc.tensor.matmul(
                                    pp[ch][m][:, :CHUNKS[ch]],
                                    W2[:, k, 128 * m : 128 * (m + 1)],
                                    hs[k][:, sl],
                                    start=(k == 0),
                                    stop=(k == 1),
                                )

                    def sq(ch):
                        qs[ch] = []
                        for m in range(2):
                            q = qsp.tile([128, 512], F32R, tag="qs",
                                         name=f"qs{ch}_{m}")
                            nc.scalar.activation(
                                out=q[:, :CHUNKS[ch]],
                                in_=pp[ch][m][:, :CHUNKS[ch]],
                                func=AT.Square,
                                bias=B2[:, m : m + 1],
                            )
                            qs[ch].append(q)

                    def colsum_ln(ch):
                        # PE: nsq = ones^T (qs0 | qs1); then ACT ln
                        cw = CHUNKS[ch]
                        nsq = nsqp.tile([1, 512], F32, tag="nsq")
                        for m in range(2):
                            nc.tensor.matmul(
                                nsq[:, :cw], onesc[:], qs[ch][m][:, :cw],
                                start=(m == 0), stop=(m == 1),
                            )
                        t = big.tile([1, 512], F32R, tag="lnt",
                                     name="lnt", bufs=2)
                        nc.scalar.activation(
                            out=t[:, :cw], in_=nsq[:, :cw], func=AT.Ln,
                            scale=float(TAU),
                        )
                        lnt[ch] = t

                    def rrep_mm(ch):
                        # PE: broadcast ln row to 128 partitions
                        cw = CHUNKS[ch]
                        r = mmp.tile([128, 512], F32, tag="mm",
                                     name=f"rr{ch}")
                        nc.tensor.matmul(
                            r[:, :cw], onesr[:], lnt[ch][:, :cw],
                            start=True, stop=True,
                        )
                        rre[ch] = r

                    def rn_exp(ch):
                        # ACT: rn = exp(-0.5*ln(tau*nsq)), bf16 [128,chunk]
                        cw = CHUNKS[ch]
                        e = big.tile([128, 512], BF16, tag="rre",
                                     name="rre", bufs=2)
                        nc.scalar.activation(
                            out=e[:, :cw], in_=rre[ch][:, :cw], func=AT.Exp,
                            scale=-0.5,
                        )
                        rre[ch] = e

                    def pnorm(ch):
                        # DVE: pn = (pp + b2) * rn, straight from PSUM
                        cw = CHUNKS[ch]
                        sl = slice(COFF[ch], COFF[ch] + cw)
                        for m in range(2):
                            nc.vector.scalar_tensor_tensor(
                                out=pn[m][:, sl], in0=pp[ch][m][:, :cw],
                                scalar=B2[:, m : m + 1], in1=rre[ch][:, :cw],
                                op0=ALU.add, op1=ALU.mult,
                            )

                    def sims(n0, n1):
                        for n in range(n0, n1):
                            a0 = SAMP * n
                            j, g = n % NJ, n // NJ
                            dst = sim[32 * j : 32 * (j + 1),
                                      64 * g : 64 * (g + 1)]
                            for k in range(2):
                                nc.tensor.matmul(
                                    dst,
                                    pn[k][:, a0 : a0 + 32],
                                    pn[k][:, a0 + 32 : a0 + 96],
                                    start=(k == 0),
                                    stop=(k == 1),
                                )

                    # ---- software-pipelined issue order ----
                    l1(0); relu(0); l2(0); sq(0)
                    l1(1); relu(1); colsum_ln(0)
                    l2(1); sq(1); rrep_mm(0); rn_exp(0); pnorm(0)
                    l1(2); relu(2); colsum_ln(1)
                    # zero the two unwritten sim slots (g=4, j=1,2)
                    nc.vector.memset(sim[32:64, 256:320], 0.0)
                    nc.vector.memset(sim[64:96, 256:320], 0.0)
                    l2(2); sq(2); rrep_mm(1); rn_exp(1); pnorm(1)
                    sims(*SIMS_AT[0]); colsum_ln(2)
                    rrep_mm(2); rn_exp(2); pnorm(2)
                    sims(*SIMS_AT[1]); sims(*SIMS_AT[2])

                    # ---- max-free InfoNCE chain on [96, 5, 64] ----
                    sim3 = sim[:].rearrange("p (g m) -> p g m", g=NG)
                    ee = big.tile([96, NG * 32], F32, name="ee")
                    nc.scalar.activation(
                        out=ee[:].rearrange("p (g m) -> p g m", g=NG),
                        in_=sim3[:, :, 32:64], func=AT.Exp,
                    )
                    s4 = big.tile([96, NG], F32, name="s4")
                    nc.vector.reduce_sum(
                        out=s4[:],
                        in_=ee[:].rearrange("p (g m) -> p g m", g=NG),
                        axis=mybir.AxisListType.X,
                    )
                    em = big.tile([96, NG * 32], F32, name="em")
                    nc.scalar.activation(
                        out=em[:].rearrange("p (g m) -> p g m", g=NG),
                        in_=sim3[:, :, 0:32], func=AT.Exp, scale=-1.0,
                    )
                    tt = big.tile([96, NG * 32], F32, name="tt")
                    nc.vector.tensor_mul(
                        out=tt[:].rearrange("p (g m) -> p g m", g=NG),
                        in0=em[:].rearrange("p (g m) -> p g m", g=NG),
                        in1=s4[:].unsqueeze(-1).broadcast_to([96, NG, 32]),
                    )
                    ctb = big.tile([96, NG * 32], F32, name="ctb")
                    nc.scalar.activation(
                        out=ctb[:], in_=tt[:], func=AT.Ln, bias=1.0,
                    )
                    rowr = big.tile([96, NG], F32, name="rowr")
                    nc.vector.reduce_sum(
                        out=rowr[:],
                        in_=ctb[:].rearrange("p (g m) -> p g m", g=NG),
                        axis=mybir.AxisListType.X,
                    )
                    lp = lpp.tile([NG, NJ], F32, tag="lp")
                    nc.tensor.matmul(
                        lp[:], rowr[:], blk4[:96, :NJ], start=True, stop=True,
                    )
                    lout = big.tile([NG, NJ], F32, name="lout")
                    nc.scalar.mul(
                        out=lout[:], in_=lp[:],
                        mul=1.0 / (NUM_SAMPLES * NUM_SAMPLES),
                    )
                    nc.sync.dma_start(
                        out=loss.rearrange("(a b) -> a b", b=NJ), in_=lout[:]
                    )

    nc.compile()
    return nc


_NC_CACHE = None


def _get_nc():
    global _NC_CACHE
    if _NC_CACHE is None:
        _NC_CACHE = _build_nc()
    return _NC_CACHE


def _host_prep(feats, w1, b1, w2, b2, anchor_inds, pos_inds, neg_inds):
    """Build the 8 per-core input maps."""
    n = feats.shape[0]
    ntot = N_CORES * NI
    ff = np.asarray(feats, dtype=np.float32).reshape(n, C, PIX)

    def flat(inds):
        inds = np.asarray(inds)
        f = inds[..., 0].astype(np.int64) * SIDE + inds[..., 1].astype(np.int64)
        if ntot > n:
            f = np.concatenate(
                [f, np.broadcast_to(f[0], (ntot - n,) + f.shape[1:])], axis=0
            )
        return f  # [ntot, 32]

    af, pf, nf = flat(anchor_inds), flat(pos_inds), flat(neg_inds)
    samp = np.concatenate([af, pf, nf], axis=1)  # [ntot, 96]
    idx = np.minimum(np.arange(ntot), n - 1)
    g = np.take_along_axis(ff[idx], samp[:, None, :], axis=2)
    # per-core channel-major [C, 1248] -> bf16 chunks [128, 2, cw]
    g = g.reshape(N_CORES, NI, C, SAMP)
    g = np.transpose(g, (0, 2, 1, 3)).reshape(N_CORES, C, STOT)
    g = g.astype(ml_dtypes.bfloat16)
    g = g.reshape(N_CORES, 2, 128, STOT)  # c = k*128 + p
    g = np.transpose(g, (0, 2, 1, 3))     # [cores, 128, 2, STOT]
    gchunks = [
        np.ascontiguousarray(g[:, :, :, COFF[ch] : COFF[ch] + CHUNKS[ch]])
        for ch in range(NCH)
    ]

    def wprep(w):
        wt = np.asarray(w, dtype=np.float32).T  # [c, d]
        wt = wt.reshape(2, 128, C)              # [k, p, d]
        return np.transpose(wt, (1, 0, 2)).astype(ml_dtypes.bfloat16)

    wcm = np.ascontiguousarray(
        np.stack([wprep(w1), wprep(w2)], axis=1)
    )  # [128, 2, 2, C]
    bcm = np.zeros((128, 8), dtype=np.float32)
    bcm[:, 0:2] = np.asarray(b1, dtype=np.float32).reshape(2, 128).T
    bcm[:, 2:4] = np.asarray(b2, dtype=np.float32).reshape(2, 128).T
    for j in range(4):
        bcm[32 * j : 32 * (j + 1), 4 + j] = 1.0

    in_maps = []
    for c in range(N_CORES):
        m = {"wcm": wcm, "bcm": bcm}
        for ch in range(NCH):
            m[f"gt{ch}"] = gchunks[ch][c]
        in_maps.append(m)
    return in_maps


def _finalize(loss_per, gt_mask):
    gt = np.asarray(gt_mask)
    area = gt.reshape(gt.shape[0], -1).sum(axis=1)
    valid = (area > NUM_SAMPLES) & (area < PIX - NUM_SAMPLES)
    n_valid = np.float32(valid.sum())
    if n_valid > 0:
        total = np.float32(np.where(valid, loss_per, 0.0).astype(np.float32).sum())
        out = total / max(n_valid, np.float32(1.0))
    else:
        out = np.float32(0.0)
    return np.float32(out * np.float32(LOSS_WEIGHT))


def kernel(feats, w1, b1, w2, b2, gt_mask, anchor_inds, pos_inds, neg_inds,
           _results_hook=None):
    nc = _get_nc()
    in_maps = _host_prep(feats, w1, b1, w2, b2, anchor_inds, pos_inds, neg_inds)
    res = run_bass_kernel_spmd(nc, in_maps, list(range(N_CORES)))
    if _results_hook is not None:
        _results_hook(res)
    parts = [res.results[c]["loss"][:NI] for c in range(N_CORES)]
    loss_per = np.concatenate(parts)[:N_INST]
    return _finalize(loss_per, gt_mask)


# revision 26
# speedup vs baseline: 1.5822x; 1.0067x over previous
"""Trainium2 Bass kernel for nn_DenseContrastLoss.

Strategy (data-parallel over instances, 8 cores x 13 instances):
  - Host: gather the 96 sampled pixel-vectors per instance (32 anchor +
    32 pos + 32 neg; indices are host-known) from feats, transpose to
    channel-major, convert to bf16, ship [3 chunks x 128 x 2 x 416] per
    core plus bf16 weights.
  - Device (per core), software-pipelined across 3 chunks so the
    in-order PE queue never waits on the DVE/ACT chain:
      L1 (bf16 matmuls, PSUM-bank alternated) -> relu+b1 (DVE)
      L2 (bf16) -> qs = Square(pp+b2) (ACT, f32r)
      colsum matmul -> ln (ACT) -> broadcast matmul -> rn = exp(-ln/2)
      (ACT, [128,chunk] bf16) -> pn = (pp+b2)*rn (DVE
      scalar_tensor_tensor, straight from PSUM)
    then 13 similarity matmuls col-tiled into one [96, 5*64] PSUM tile
    (instance n -> partition block n%3, col group n//3), and a max-free
    InfoNCE chain: term = ln(1 + sum_m' exp(an) * exp(-ap)) (the
    reference's max-subtraction cancels algebraically), finished by a
    [96,5]x[96,3] block-sum matmul -> 15 floats out.
  - Host: validity mask from gt_mask areas, masked mean, * LOSS_WEIGHT.
"""

import sys

import numpy as np

if "/opt/trn_rl_repo" not in sys.path:
    sys.path.insert(0, "/opt/trn_rl_repo")

import ml_dtypes

import concourse.bass as bass
import concourse.tile as tile
from concourse import bacc, mybir
from concourse.bass_utils import run_bass_kernel_spmd

F32 = mybir.dt.float32
F32R = mybir.dt.float32r
BF16 = mybir.dt.bfloat16

TAU = 0.07
LOSS_WEIGHT = 1.2
NUM_SAMPLES = 32
C = 256
SIDE = 28
PIX = SIDE * SIDE  # 784
N_INST = 100
N_CORES = 8
NI = 13                      # instances per core (8*13 = 104 >= 100)
SAMP = 3 * NUM_SAMPLES       # 96 sampled pixels per instance
STOT = NI * SAMP             # 1248
# uneven chunks (each <= 512, the PSUM fp32 bank limit): a small last
# chunk shortens the serial norm->sim->loss tail
CHUNKS = [512, 448, 288]
COFF = [0, 512, 960]
NCH = len(CHUNKS)
# instances fully covered once chunk ch's pn is written
SIMS_AT = [(0, 5), (5, 10), (10, 13)]
NJ, NG = 3, 5                # sim packing: partition blocks x col groups
NWARM = 16                   # PE warm-up matmuls during the input DMA


def _build_nc():
    nc = bacc.Bacc("TRN2", target_bir_lowering=False)
    gts = [nc.declare_dram_parameter(f"gt{ch}", [128, 2, CHUNKS[ch]], BF16,
                                     isOutput=False)
           for ch in range(NCH)]
    wcm = nc.declare_dram_parameter("wcm", [128, 2, 2, C], BF16,
                                    isOutput=False)
    bcm = nc.declare_dram_parameter("bcm", [128, 8], F32, isOutput=False)
    loss = nc.declare_dram_parameter("loss", [15], F32, isOutput=True)

    AT = mybir.ActivationFunctionType
    ALU = mybir.AluOpType
    PSUM = bass.MemorySpace.PSUM

    with tile.TileContext(nc) as tc:
        with tc.tile_pool(name="singles", bufs=1) as singles:
            # gather chunks on the sync ring; weights/biases on the ACT
            # ring (issued before the ACT table load) so both rings issue
            # descriptors in parallel
            WC = singles.tile([128, 2, 2, C], BF16)
            gch = [singles.tile([128, 2, CHUNKS[ch]], BF16, name=f"g{ch}")
                   for ch in range(NCH)]
            for ch in range(NCH):
                nc.sync.dma_start(out=gch[ch][:], in_=gts[ch][:, :, :])
            nc.scalar.dma_start(out=WC[:], in_=wcm[:, :, :, :])
            BC = singles.tile([128, 8], F32)
            nc.scalar.dma_start(out=BC[:], in_=bcm[:, :])
            # ACT table set (covers exp/ln/square/relu/copy), after the
            # ACT-ring DMA issues but well before the first activation
            nc.scalar.add_instruction(
                mybir.InstLoadActFuncSet(
                    name=nc.get_next_instruction_name(),
                    ins=[],
                    outs=[],
                    act_func_set_id=6,  # natural_log_exp_and_others
                )
            )
            W1, W2 = WC[:, 0], WC[:, 1]
            B1, B2, blk4 = BC[:, 0:2], BC[:, 2:4], BC[:, 4:8]

            onesrf = singles.tile([1, 128], F32)
            nc.vector.memset(onesrf[:], 1.0)
            onescf = singles.tile([128, 1], F32)
            nc.vector.memset(onescf[:], 1.0)
            onesr = singles.tile([1, 128], F32R)
            nc.scalar.copy(out=onesr[:], in_=onesrf[:])
            onesc = singles.tile([128, 1], F32R)
            nc.scalar.copy(out=onesc[:], in_=onescf[:])

            with tc.tile_pool(name="big", bufs=1) as big:
                hs = [big.tile([128, STOT], BF16, name=f"hs{m}")
                      for m in range(2)]
                pn = [big.tile([128, STOT], BF16, name=f"pn{m}")
                      for m in range(2)]

                with (
                    tc.tile_pool(name="mmp", bufs=6, space=PSUM) as mmp,
                    tc.tile_pool(name="nsqp", bufs=1, space=PSUM) as nsqp,
                    tc.tile_pool(name="simp", bufs=1, space=PSUM) as simp,
                    tc.tile_pool(name="qsp", bufs=4) as qsp,
                ):
                    # one bank: sims in cols 0-319, final loss in 320-322
                    sim = simp.tile([96, 512], F32, tag="sim")

                    # PE warm-up during the input-DMA window: ramps HAM to
                    # K=8/8 before the real matmuls arrive.  Writes land in
                    # the sim bank and are overwritten later (start=True).
                    warm = singles.tile([128, 416], BF16, name="warm")
                    nc.vector.memset(warm[:], 1.0)
                    for _ in range(NWARM):
                        nc.tensor.matmul(
                            sim[:96, :320], warm[:, :96], warm[:, :320],
                            start=True, stop=True,
                        )

                    hp = {}
                    pp = {}
                    qs = {}
                    lnt = {}
                    rre = {}

                    def l1(ch):
                        cw = CHUNKS[ch]
                        hp[ch] = [mmp.tile([128, 512], F32, tag="mm",
                                           name=f"hp{ch}_{m}")
                                  for m in range(2)]
                        for k in range(2):
                            for m in range(2):
                                nc.tensor.matmul(
                                    hp[ch][m][:, :cw],
                                    W1[:, k, 128 * m : 128 * (m + 1)],
                                    gch[ch][:, k, :],
                                    start=(k == 0),
                                    stop=(k == 1),
                                )

                    def relu(ch):
                        sl = slice(COFF[ch], COFF[ch] + CHUNKS[ch])
                        for m in range(2):
                            nc.vector.tensor_scalar(
                                out=hs[m][:, sl], in0=hp[ch][m][:, :CHUNKS[ch]],
                                scalar1=B1[:, m : m + 1], scalar2=0.0,
                                op0=ALU.add, op1=ALU.max,
                            )

                    def l2(ch):
                        sl = slice(COFF[ch], COFF[ch] + CHUNKS[ch])
                        pp[ch] = [mmp.tile([128, 512], F32, tag="mm",
                                           name=f"pp{ch}_{m}")
                                  for m in range(2)]
                        for k in range(2):
                            for m in range(2):
                                nc.tensor.matmul(
                                    pp[ch][m][:, :CHUNKS[ch]],
                                    W2[:, k, 128 * m : 128 * (m + 1)],
                                    hs[k][:, sl],
                                    start=(k == 0),
                                    stop=(k == 1),
                                )

                    def sq(ch):
                        qs[ch] = []
                        for m in range(2):
                            q = qsp.tile([128, 512], F32R, tag="qs",
                                         name=f"qs{ch}_{m}")
                            nc.scalar.activation(
                                out=q[:, :CHUNKS[ch]],
                                in_=pp[ch][m][:, :CHUNKS[ch]],
                                func=AT.Square,
                                bias=B2[:, m : m + 1],
                            )
                            qs[ch].append(q)

                    def colsum_ln(ch):
                        # PE: nsq = ones^T (qs0 | qs1); then ACT ln
                        cw = CHUNKS[ch]
                        nsq = nsqp.tile([1, 512], F32, tag="nsq")
                        for m in range(2):
                            nc.tensor.matmul(
                                nsq[:, :cw], onesc[:], qs[ch][m][:, :cw],
                                start=(m == 0), stop=(m == 1),
                            )
                        t = big.tile([1, 512], F32R, tag="lnt",
                                     name="lnt", bufs=2)
                        nc.scalar.activation(
                            out=t[:, :cw], in_=nsq[:, :cw], func=AT.Ln,
                            scale=float(TAU),
                        )
                        lnt[ch] = t

                    def rrep_mm(ch):
                        # PE: broadcast ln row to 128 partitions
                        cw = CHUNKS[ch]
                        r = mmp.tile([128, 512], F32, tag="mm",
                                     name=f"rr{ch}")
                        nc.tensor.matmul(
                            r[:, :cw], onesr[:], lnt[ch][:, :cw],
                            start=True, stop=True,
                        )
                        rre[ch] = r

                    def rn_exp(ch):
                        # ACT: rn = exp(-0.5*ln(tau*nsq)), bf16 [128,chunk]
                        cw = CHUNKS[ch]
                        e = big.tile([128, 512], BF16, tag="rre",
                                     name="rre", bufs=2)
                        nc.scalar.activation(
                            out=e[:, :cw], in_=rre[ch][:, :cw], func=AT.Exp,
                            scale=-0.5,
                        )
                        rre[ch] = e

                    def pnorm(ch):
                        # DVE: pn = (pp + b2) * rn, straight from PSUM
                        cw = CHUNKS[ch]
                        sl = slice(COFF[ch], COFF[ch] + cw)
                        for m in range(2):
                            nc.vector.scalar_tensor_tensor(
                                out=pn[m][:, sl], in0=pp[ch][m][:, :cw],
                                scalar=B2[:, m : m + 1], in1=rre[ch][:, :cw],
                                op0=ALU.add, op1=ALU.mult,
                            )

                    def sims(n0, n1):
                        for n in range(n0, n1):
                            a0 = SAMP * n
                            j, g = n % NJ, n // NJ
                            dst = sim[32 * j : 32 * (j + 1),
                                      64 * g : 64 * (g + 1)]
                            for k in range(2):
                                nc.tensor.matmul(
                                    dst,
                                    pn[k][:, a0 : a0 + 32],
                                    pn[k][:, a0 + 32 : a0 + 96],
                                    start=(k == 0),
                                    stop=(k == 1),
                                )

                    # ---- software-pipelined issue order ----
                    l1(0); relu(0)
                    l1(1); relu(1)
                    l2(0); sq(0); colsum_ln(0)
                    l2(1); sq(1); rrep_mm(0); rn_exp(0); pnorm(0)
                    l1(2); relu(2); colsum_ln(1)
                    # zero the two unwritten sim slots (g=4, j=1,2)
                    nc.vector.memset(sim[32:64, 256:320], 0.0)
                    nc.vector.memset(sim[64:96, 256:320], 0.0)
                    l2(2); sq(2); rrep_mm(1); rn_exp(1); pnorm(1)
                    sims(*SIMS_AT[0]); colsum_ln(2)
                    rrep_mm(2); rn_exp(2); pnorm(2)
                    sims(*SIMS_AT[1]); sims(*SIMS_AT[2])

                    # ---- max-free InfoNCE chain on [96, 5, 64] ----
                    sim3 = sim[:, : NG * 64].rearrange(
                        "p (g m) -> p g m", g=NG
                    )
                    ee = big.tile([96, NG * 32], F32, name="ee")
                    nc.scalar.activation(
                        out=ee[:].rearrange("p (g m) -> p g m", g=NG),
                        in_=sim3[:, :, 32:64], func=AT.Exp,
                    )
                    s4 = big.tile([96, NG], F32, name="s4")
                    nc.vector.reduce_sum(
                        out=s4[:],
                        in_=ee[:].rearrange("p (g m) -> p g m", g=NG),
                        axis=mybir.AxisListType.X,
                    )
                    em = big.tile([96, NG * 32], F32, name="em")
                    nc.scalar.activation(
                        out=em[:].rearrange("p (g m) -> p g m", g=NG),
                        in_=sim3[:, :, 0:32], func=AT.Exp, scale=-1.0,
                    )
                    tt = big.tile([96, NG * 32], F32, name="tt")
                    nc.vector.tensor_mul(
                        out=tt[:].rearrange("p (g m) -> p g m", g=NG),
                        in0=em[:].rearrange("p (g m) -> p g m", g=NG),
                        in1=s4[:].unsqueeze(-1).broadcast_to([96, NG, 32]),
                    )
                    ctb = big.tile([96, NG * 32], F32, name="ctb")
                    nc.scalar.activation(
                        out=ctb[:], in_=tt[:], func=AT.Ln, bias=1.0,
                    )
                    rowr = big.tile([96, NG], F32, name="rowr")
                    nc.vector.reduce_sum(
                        out=rowr[:],
                        in_=ctb[:].rearrange("p (g m) -> p g m", g=NG),
                        axis=mybir.AxisListType.X,
                    )
                    lp = sim[0:NG, 320 : 320 + NJ]
                    nc.tensor.matmul(
                        lp, rowr[:], blk4[:96, :NJ], start=True, stop=True,
                    )
                    lout = big.tile([NG, NJ], F32, name="lout")
                    nc.vector.tensor_scalar_mul(
                        out=lout[:], in0=lp,
                        scalar1=1.0 / (NUM_SAMPLES * NUM_SAMPLES),
                    )
                    nc.sync.dma_start(
                        out=loss.rearrange("(a b) -> a b", b=NJ), in_=lout[:]
                    )

    nc.compile()
    return nc


_NC_CACHE = None


def _get_nc():
    global _NC_CACHE
    if _NC_CACHE is None:
        _NC_CACHE = _build_nc()
    return _NC_CACHE


def _host_prep(feats, w1, b1, w2, b2, anchor_inds, pos_inds, neg_inds):
    """Build the 8 per-core input maps."""
    n = feats.shape[0]
    ntot = N_CORES * NI
    ff = np.asarray(feats, dtype=np.float32).reshape(n, C, PIX)

    def flat(inds):
        inds = np.asarray(inds)
        f = inds[..., 0].astype(np.int64) * SIDE + inds[..., 1].astype(np.int64)
        if ntot > n:
            f = np.concatenate(
                [f, np.broadcast_to(f[0], (ntot - n,) + f.shape[1:])], axis=0
            )
        return f  # [ntot, 32]

    af, pf, nf = flat(anchor_inds), flat(pos_inds), flat(neg_inds)
    samp = np.concatenate([af, pf, nf], axis=1)  # [ntot, 96]
    idx = np.minimum(np.arange(ntot), n - 1)
    g = np.take_along_axis(ff[idx], samp[:, None, :], axis=2)
    # per-core channel-major [C, 1248] -> bf16 chunks [128, 2, cw]
    g = g.reshape(N_CORES, NI, C, SAMP)
    g = np.transpose(g, (0, 2, 1, 3)).reshape(N_CORES, C, STOT)
    g = g.astype(ml_dtypes.bfloat16)
    g = g.reshape(N_CORES, 2, 128, STOT)  # c = k*128 + p
    g = np.transpose(g, (0, 2, 1, 3))     # [cores, 128, 2, STOT]
    gchunks = [
        np.ascontiguousarray(g[:, :, :, COFF[ch] : COFF[ch] + CHUNKS[ch]])
        for ch in range(NCH)
    ]

    def wprep(w):
        wt = np.asarray(w, dtype=np.float32).T  # [c, d]
        wt = wt.reshape(2, 128, C)              # [k, p, d]
        return np.transpose(wt, (1, 0, 2)).astype(ml_dtypes.bfloat16)

    wcm = np.ascontiguousarray(
        np.stack([wprep(w1), wprep(w2)], axis=1)
    )  # [128, 2, 2, C]
    bcm = np.zeros((128, 8), dtype=np.float32)
    bcm[:, 0:2] = np.asarray(b1, dtype=np.float32).reshape(2, 128).T
    bcm[:, 2:4] = np.asarray(b2, dtype=np.float32).reshape(2, 128).T
    for j in range(4):
        bcm[32 * j : 32 * (j + 1), 4 + j] = 1.0

    in_maps = []
    for c in range(N_CORES):
        m = {"wcm": wcm, "bcm": bcm}
        for ch in range(NCH):
            m[f"gt{ch}"] = gchunks[ch][c]
        in_maps.append(m)
    return in_maps


def _finalize(loss_per, gt_mask):
    gt = np.asarray(gt_mask)
    area = gt.reshape(gt.shape[0], -1).sum(axis=1)
    valid = (area > NUM_SAMPLES) & (area < PIX - NUM_SAMPLES)
    n_valid = np.float32(valid.sum())
    if n_valid > 0:
        total = np.float32(np.where(valid, loss_per, 0.0).astype(np.float32).sum())
        out = total / max(n_valid, np.float32(1.0))
    else:
        out = np.float32(0.0)
    return np.float32(out * np.float32(LOSS_WEIGHT))


def kernel(feats, w1, b1, w2, b2, gt_mask, anchor_inds, pos_inds, neg_inds,
           _results_hook=None):
    nc = _get_nc()
    in_maps = _host_prep(feats, w1, b1, w2, b2, anchor_inds, pos_inds, neg_inds)
    res = run_bass_kernel_spmd(nc, in_maps, list(range(N_CORES)))
    if _results_hook is not None:
        _results_hook(res)
    parts = [res.results[c]["loss"][:NI] for c in range(N_CORES)]
    loss_per = np.concatenate(parts)[:N_INST]
    return _finalize(loss_per, gt_mask)


# revision 30
# speedup vs baseline: 1.7310x; 1.0940x over previous
"""Trainium2 Bass kernel for nn_DenseContrastLoss.

Strategy (data-parallel over instances, 8 cores x 13 instances):
  - Host: gather the 96 sampled pixel-vectors per instance (32 anchor +
    32 pos + 32 neg; indices are host-known) from feats, transpose to
    channel-major, convert to bf16, ship [3 chunks x 128 x 2 x 416] per
    core plus bf16 weights.
  - Device (per core), software-pipelined across 3 chunks so the
    in-order PE queue never waits on the DVE/ACT chain:
      L1 (bf16 matmuls, PSUM-bank alternated) -> relu+b1 (DVE)
      L2 (bf16) -> qs = Square(pp+b2) (ACT, f32r)
      colsum matmul -> ln (ACT) -> broadcast matmul -> rn = exp(-ln/2)
      (ACT, [128,chunk] bf16) -> pn = (pp+b2)*rn (DVE
      scalar_tensor_tensor, straight from PSUM)
    then 13 similarity matmuls col-tiled into one [96, 5*64] PSUM tile
    (instance n -> partition block n%3, col group n//3), and a max-free
    InfoNCE chain: term = ln(1 + sum_m' exp(an) * exp(-ap)) (the
    reference's max-subtraction cancels algebraically), finished by a
    [96,5]x[96,3] block-sum matmul -> 15 floats out.
  - Host: validity mask from gt_mask areas, masked mean, * LOSS_WEIGHT.
"""

import sys

import numpy as np

if "/opt/trn_rl_repo" not in sys.path:
    sys.path.insert(0, "/opt/trn_rl_repo")

import ml_dtypes

import concourse.bass as bass
import concourse.tile as tile
from concourse import bacc, mybir
from concourse.bass_utils import run_bass_kernel_spmd

F32 = mybir.dt.float32
F32R = mybir.dt.float32r
BF16 = mybir.dt.bfloat16

TAU = 0.07
LOSS_WEIGHT = 1.2
NUM_SAMPLES = 32
C = 256
SIDE = 28
PIX = SIDE * SIDE  # 784
N_INST = 100
N_CORES = 8
NI = 13                      # instances per core (8*13 = 104 >= 100)
SAMP = 3 * NUM_SAMPLES       # 96 sampled pixels per instance
STOT = NI * SAMP             # 1248
# uneven chunks (each <= 512, the PSUM fp32 bank limit): a small last
# chunk shortens the serial norm->sim->loss tail
CHUNKS = [512, 448, 288]
COFF = [0, 512, 960]
NCH = len(CHUNKS)
# instances fully covered once chunk ch's pn is written
SIMS_AT = [(0, 5), (5, 10), (10, 13)]
NJ, NG = 3, 5                # sim packing: partition blocks x col groups
NWARM = 16                   # PE warm-up matmuls during the input DMA


def _build_nc():
    nc = bacc.Bacc("TRN2", target_bir_lowering=False)
    gts = [nc.declare_dram_parameter(f"gt{ch}", [128, 2, CHUNKS[ch]], BF16,
                                     isOutput=False)
           for ch in range(NCH)]
    wcm = nc.declare_dram_parameter("wcm", [128, 2, 2, C], BF16,
                                    isOutput=False)
    bcm = nc.declare_dram_parameter("bcm", [128, 8], F32, isOutput=False)
    loss = nc.declare_dram_parameter("loss", [15], F32, isOutput=True)

    AT = mybir.ActivationFunctionType
    ALU = mybir.AluOpType
    PSUM = bass.MemorySpace.PSUM

    with tile.TileContext(nc) as tc:
        with tc.tile_pool(name="singles", bufs=1) as singles:
            # weights + gather chunks on the sync ring (weights first:
            # they gate the first LDWEIGHTS); tiny biases on the ACT ring
            WC = singles.tile([128, 2, 2, C], BF16)
            nc.sync.dma_start(out=WC[:], in_=wcm[:, :, :, :])
            gch = [singles.tile([128, 2, CHUNKS[ch]], BF16, name=f"g{ch}")
                   for ch in range(NCH)]
            for ch in range(NCH):
                nc.sync.dma_start(out=gch[ch][:], in_=gts[ch][:, :, :])
            BC = singles.tile([128, 8], F32)
            nc.scalar.dma_start(out=BC[:], in_=bcm[:, :])
            # ACT table set (covers exp/ln/square/relu/copy), after the
            # ACT-ring DMA issues but well before the first activation
            nc.scalar.add_instruction(
                mybir.InstLoadActFuncSet(
                    name=nc.get_next_instruction_name(),
                    ins=[],
                    outs=[],
                    act_func_set_id=6,  # natural_log_exp_and_others
                )
            )
            W1, W2 = WC[:, 0], WC[:, 1]
            B1, B2, blk4 = BC[:, 0:2], BC[:, 2:4], BC[:, 4:8]

            onesrf = singles.tile([1, 128], F32)
            nc.vector.memset(onesrf[:], 1.0)
            onescf = singles.tile([128, 1], F32)
            nc.vector.memset(onescf[:], 1.0)
            onesr = singles.tile([1, 128], F32R)
            nc.scalar.copy(out=onesr[:], in_=onesrf[:])
            onesc = singles.tile([128, 1], F32R)
            nc.scalar.copy(out=onesc[:], in_=onescf[:])

            with tc.tile_pool(name="big", bufs=1) as big:
                hs = [big.tile([128, STOT], BF16, name=f"hs{m}")
                      for m in range(2)]
                pn = [big.tile([128, STOT], BF16, name=f"pn{m}")
                      for m in range(2)]

                with (
                    tc.tile_pool(name="mmp", bufs=6, space=PSUM) as mmp,
                    tc.tile_pool(name="nsqp", bufs=1, space=PSUM) as nsqp,
                    tc.tile_pool(name="simp", bufs=1, space=PSUM) as simp,
                    tc.tile_pool(name="qsp", bufs=4) as qsp,
                ):
                    # one bank: sims in cols 0-319, final loss in 320-322
                    sim = simp.tile([96, 512], F32, tag="sim")

                    # PE warm-up during the input-DMA window: ramps HAM to
                    # K=8/8 before the real matmuls arrive.  Writes land in
                    # the sim bank and are overwritten later (start=True).
                    warm = singles.tile([128, 416], BF16, name="warm")
                    nc.vector.memset(warm[:], 1.0)
                    for _ in range(NWARM):
                        nc.tensor.matmul(
                            sim[:96, :320], warm[:, :96], warm[:, :320],
                            start=True, stop=True,
                        )

                    hp = {}
                    pp = {}
                    qs = {}
                    lnt = {}
                    rre = {}

                    def l1(ch):
                        cw = CHUNKS[ch]
                        hp[ch] = [mmp.tile([128, 512], F32, tag="mm",
                                           name=f"hp{ch}_{m}")
                                  for m in range(2)]
                        for k in range(2):
                            for m in range(2):
                                nc.tensor.matmul(
                                    hp[ch][m][:, :cw],
                                    W1[:, k, 128 * m : 128 * (m + 1)],
                                    gch[ch][:, k, :],
                                    start=(k == 0),
                                    stop=(k == 1),
                                )

                    def relu(ch):
                        sl = slice(COFF[ch], COFF[ch] + CHUNKS[ch])
                        for m in range(2):
                            nc.vector.tensor_scalar(
                                out=hs[m][:, sl], in0=hp[ch][m][:, :CHUNKS[ch]],
                                scalar1=B1[:, m : m + 1], scalar2=0.0,
                                op0=ALU.add, op1=ALU.max,
                            )

                    def l2(ch):
                        sl = slice(COFF[ch], COFF[ch] + CHUNKS[ch])
                        pp[ch] = [mmp.tile([128, 512], F32, tag="mm",
                                           name=f"pp{ch}_{m}")
                                  for m in range(2)]
                        for k in range(2):
                            for m in range(2):
                                nc.tensor.matmul(
                                    pp[ch][m][:, :CHUNKS[ch]],
                                    W2[:, k, 128 * m : 128 * (m + 1)],
                                    hs[k][:, sl],
                                    start=(k == 0),
                                    stop=(k == 1),
                                )

                    def sq(ch):
                        qs[ch] = []
                        for m in range(2):
                            q = qsp.tile([128, 512], F32R, tag="qs",
                                         name=f"qs{ch}_{m}")
                            nc.scalar.activation(
                                out=q[:, :CHUNKS[ch]],
                                in_=pp[ch][m][:, :CHUNKS[ch]],
                                func=AT.Square,
                                bias=B2[:, m : m + 1],
                            )
                            qs[ch].append(q)

                    nsqs = {}

                    def colsum(ch):
                        # PE: nsq = ones^T (qs0 | qs1)
                        cw = CHUNKS[ch]
                        nsq = nsqp.tile([1, 512], F32, tag="nsq")
                        for m in range(2):
                            nc.tensor.matmul(
                                nsq[:, :cw], onesc[:], qs[ch][m][:, :cw],
                                start=(m == 0), stop=(m == 1),
                            )
                        nsqs[ch] = nsq

                    def ln_op(ch):
                        cw = CHUNKS[ch]
                        t = big.tile([1, 512], F32R, tag="lnt",
                                     name="lnt", bufs=2)
                        nc.scalar.activation(
                            out=t[:, :cw], in_=nsqs[ch][:, :cw], func=AT.Ln,
                            scale=float(TAU),
                        )
                        lnt[ch] = t

                    def rrep_mm(ch):
                        # PE: broadcast ln row to 128 partitions
                        cw = CHUNKS[ch]
                        r = mmp.tile([128, 512], F32, tag="mm",
                                     name=f"rr{ch}")
                        nc.tensor.matmul(
                            r[:, :cw], onesr[:], lnt[ch][:, :cw],
                            start=True, stop=True,
                        )
                        rre[ch] = r

                    def rn_exp(ch):
                        # ACT: rn = exp(-0.5*ln(tau*nsq)), bf16 [128,chunk]
                        cw = CHUNKS[ch]
                        e = big.tile([128, 512], BF16, tag="rre",
                                     name="rre", bufs=2)
                        nc.scalar.activation(
                            out=e[:, :cw], in_=rre[ch][:, :cw], func=AT.Exp,
                            scale=-0.5,
                        )
                        rre[ch] = e

                    def pnorm(ch):
                        # DVE: pn = (pp + b2) * rn, straight from PSUM
                        cw = CHUNKS[ch]
                        sl = slice(COFF[ch], COFF[ch] + cw)
                        for m in range(2):
                            nc.vector.scalar_tensor_tensor(
                                out=pn[m][:, sl], in0=pp[ch][m][:, :cw],
                                scalar=B2[:, m : m + 1], in1=rre[ch][:, :cw],
                                op0=ALU.add, op1=ALU.mult,
                            )

                    def sims(n0, n1):
                        for n in range(n0, n1):
                            a0 = SAMP * n
                            j, g = n % NJ, n // NJ
                            dst = sim[32 * j : 32 * (j + 1),
                                      64 * g : 64 * (g + 1)]
                            for k in range(2):
                                nc.tensor.matmul(
                                    dst,
                                    pn[k][:, a0 : a0 + 32],
                                    pn[k][:, a0 + 32 : a0 + 96],
                                    start=(k == 0),
                                    stop=(k == 1),
                                )

                    # ---- max-free InfoNCE chain, split by col groups so
                    # the first part overlaps the last chunk's norm ----
                    sim3 = sim[:, : NG * 64].rearrange(
                        "p (g m) -> p g m", g=NG
                    )
                    ee = big.tile([96, NG * 32], F32, name="ee")
                    s4 = big.tile([96, NG], F32, name="s4")
                    em = big.tile([96, NG * 32], F32, name="em")
                    tt = big.tile([96, NG * 32], F32, name="tt")
                    ctb = big.tile([96, NG * 32], F32, name="ctb")
                    rowr = big.tile([96, NG], F32, name="rowr")

                    def chain(g0, g1):
                        gn = g1 - g0
                        c0, c1 = 32 * g0, 32 * g1
                        ee3 = ee[:, c0:c1].rearrange("p (g m) -> p g m", g=gn)
                        em3 = em[:, c0:c1].rearrange("p (g m) -> p g m", g=gn)
                        nc.scalar.activation(
                            out=ee3, in_=sim3[:, g0:g1, 32:64], func=AT.Exp,
                        )
                        nc.vector.reduce_sum(
                            out=s4[:, g0:g1], in_=ee3,
                            axis=mybir.AxisListType.X,
                        )
                        nc.scalar.activation(
                            out=em3, in_=sim3[:, g0:g1, 0:32], func=AT.Exp,
                            scale=-1.0,
                        )
                        nc.vector.tensor_mul(
                            out=tt[:, c0:c1].rearrange(
                                "p (g m) -> p g m", g=gn
                            ),
                            in0=em3,
                            in1=s4[:, g0:g1].unsqueeze(-1).broadcast_to(
                                [96, gn, 32]
                            ),
                        )
                        nc.scalar.activation(
                            out=ctb[:, c0:c1], in_=tt[:, c0:c1], func=AT.Ln,
                            bias=1.0,
                        )
                        nc.vector.reduce_sum(
                            out=rowr[:, g0:g1],
                            in_=ctb[:, c0:c1].rearrange(
                                "p (g m) -> p g m", g=gn
                            ),
                            axis=mybir.AxisListType.X,
                        )

                    # ---- software-pipelined issue order ----
                    l1(0); relu(0)
                    l1(1); relu(1)
                    l1(2); relu(2)
                    l2(0); sq(0); colsum(0); ln_op(0)
                    l2(1); sq(1); rrep_mm(0); rn_exp(0); pnorm(0)
                    # zero the two unwritten sim slots (g=4, j=1,2)
                    nc.vector.memset(sim[32:64, 256:320], 0.0)
                    nc.vector.memset(sim[64:96, 256:320], 0.0)
                    colsum(1)
                    l2(2); ln_op(1); sq(2)
                    rrep_mm(1); rn_exp(1); pnorm(1)
                    sims(0, 5); sims(5, 10)
                    colsum(2); ln_op(2); rrep_mm(2); rn_exp(2); pnorm(2)
                    chain(0, 3)          # instances 0-8, overlaps chunk 2
                    sims(10, NI)
                    chain(3, NG)         # instances 9-12

                    lp = sim[0:NG, 320 : 320 + NJ]
                    nc.tensor.matmul(
                        lp, rowr[:], blk4[:96, :NJ], start=True, stop=True,
                    )
                    lout = big.tile([NG, NJ], F32, name="lout")
                    nc.vector.tensor_scalar_mul(
                        out=lout[:], in0=lp,
                        scalar1=1.0 / (NUM_SAMPLES * NUM_SAMPLES),
                    )
                    nc.sync.dma_start(
                        out=loss.rearrange("(a b) -> a b", b=NJ), in_=lout[:]
                    )

    nc.compile()
    return nc


_NC_CACHE = None


def _get_nc():
    global _NC_CACHE
    if _NC_CACHE is None:
        _NC_CACHE = _build_nc()
    return _NC_CACHE


def _host_prep(feats, w1, b1, w2, b2, anchor_inds, pos_inds, neg_inds):
    """Build the 8 per-core input maps."""
    n = feats.shape[0]
    ntot = N_CORES * NI
    ff = np.asarray(feats, dtype=np.float32).reshape(n, C, PIX)

    def flat(inds):
        inds = np.asarray(inds)
        f = inds[..., 0].astype(np.int64) * SIDE + inds[..., 1].astype(np.int64)
        if ntot > n:
            f = np.concatenate(
                [f, np.broadcast_to(f[0], (ntot - n,) + f.shape[1:])], axis=0
            )
        return f  # [ntot, 32]

    af, pf, nf = flat(anchor_inds), flat(pos_inds), flat(neg_inds)
    samp = np.concatenate([af, pf, nf], axis=1)  # [ntot, 96]
    idx = np.minimum(np.arange(ntot), n - 1)
    g = np.take_along_axis(ff[idx], samp[:, None, :], axis=2)
    # per-core channel-major [C, 1248] -> bf16 chunks [128, 2, cw]
    g = g.reshape(N_CORES, NI, C, SAMP)
    g = np.transpose(g, (0, 2, 1, 3)).reshape(N_CORES, C, STOT)
    g = g.astype(ml_dtypes.bfloat16)
    g = g.reshape(N_CORES, 2, 128, STOT)  # c = k*128 + p
    g = np.transpose(g, (0, 2, 1, 3))     # [cores, 128, 2, STOT]
    gchunks = [
        np.ascontiguousarray(g[:, :, :, COFF[ch] : COFF[ch] + CHUNKS[ch]])
        for ch in range(NCH)
    ]

    def wprep(w):
        wt = np.asarray(w, dtype=np.float32).T  # [c, d]
        wt = wt.reshape(2, 128, C)              # [k, p, d]
        return np.transpose(wt, (1, 0, 2)).astype(ml_dtypes.bfloat16)

    wcm = np.ascontiguousarray(
        np.stack([wprep(w1), wprep(w2)], axis=1)
    )  # [128, 2, 2, C]
    bcm = np.zeros((128, 8), dtype=np.float32)
    bcm[:, 0:2] = np.asarray(b1, dtype=np.float32).reshape(2, 128).T
    bcm[:, 2:4] = np.asarray(b2, dtype=np.float32).reshape(2, 128).T
    for j in range(4):
        bcm[32 * j : 32 * (j + 1), 4 + j] = 1.0

    in_maps = []
    for c in range(N_CORES):
        m = {"wcm": wcm, "bcm": bcm}
        for ch in range(NCH):
            m[f"gt{ch}"] = gchunks[ch][c]
        in_maps.append(m)
    return in_maps


def _finalize(loss_per, gt_mask):
    gt = np.asarray(gt_mask)
    area = gt.reshape(gt.shape[0], -1).sum(axis=1)
    valid = (area > NUM_SAMPLES) & (area < PIX - NUM_SAMPLES)
    n_valid = np.float32(valid.sum())
    if n_valid > 0:
        total = np.float32(np.where(valid, loss_per, 0.0).astype(np.float32).sum())
        out = total / max(n_valid, np.float32(1.0))
    else:
        out = np.float32(0.0)
    return np.float32(out * np.float32(LOSS_WEIGHT))


def kernel(feats, w1, b1, w2, b2, gt_mask, anchor_inds, pos_inds, neg_inds,
           _results_hook=None):
    nc = _get_nc()
    in_maps = _host_prep(feats, w1, b1, w2, b2, anchor_inds, pos_inds, neg_inds)
    res = run_bass_kernel_spmd(nc, in_maps, list(range(N_CORES)))
    if _results_hook is not None:
        _results_hook(res)
    parts = [res.results[c]["loss"][:NI] for c in range(N_CORES)]
    loss_per = np.concatenate(parts)[:N_INST]
    return _finalize(loss_per, gt_mask)


# revision 37
# speedup vs baseline: 1.8090x; 1.0451x over previous
"""Trainium2 Bass kernel for nn_DenseContrastLoss.

Strategy (data-parallel over instances, 8 cores x 13 instances):
  - Host: gather the 96 sampled pixel-vectors per instance (32 anchor +
    32 pos + 32 neg; indices are host-known) from feats, transpose to
    channel-major, convert to bf16, ship [3 chunks x 128 x 2 x 416] per
    core plus bf16 weights.
  - Device (per core), software-pipelined across 3 chunks so the
    in-order PE queue never waits on the DVE/ACT chain:
      L1 (bf16 matmuls, PSUM-bank alternated) -> relu+b1 (DVE)
      L2 (bf16) -> qs = Square(pp+b2) (ACT, f32r)
      colsum matmul -> ln (ACT) -> broadcast matmul -> rn = exp(-ln/2)
      (ACT, [128,chunk] bf16) -> pn = (pp+b2)*rn (DVE
      scalar_tensor_tensor, straight from PSUM)
    then 13 similarity matmuls col-tiled into one [96, 5*64] PSUM tile
    (instance n -> partition block n%3, col group n//3), and a max-free
    InfoNCE chain: term = ln(1 + sum_m' exp(an) * exp(-ap)) (the
    reference's max-subtraction cancels algebraically), finished by a
    [96,5]x[96,3] block-sum matmul -> 15 floats out.
  - Host: validity mask from gt_mask areas, masked mean, * LOSS_WEIGHT.
"""

import sys

import numpy as np

if "/opt/trn_rl_repo" not in sys.path:
    sys.path.insert(0, "/opt/trn_rl_repo")

import ml_dtypes

import concourse.bass as bass
import concourse.tile as tile
from concourse import bacc, mybir
from concourse.bass_utils import run_bass_kernel_spmd

F32 = mybir.dt.float32
F32R = mybir.dt.float32r
BF16 = mybir.dt.bfloat16

TAU = 0.07
LOSS_WEIGHT = 1.2
NUM_SAMPLES = 32
C = 256
SIDE = 28
PIX = SIDE * SIDE  # 784
N_INST = 100
N_CORES = 8
NI = 13                      # instances per core (8*13 = 104 >= 100)
SAMP = 3 * NUM_SAMPLES       # 96 sampled pixels per instance
STOT = NI * SAMP             # 1248
# uneven chunks (each <= 512, the PSUM fp32 bank limit): a small last
# chunk shortens the serial norm->sim->loss tail
CHUNKS = [512, 448, 288]
COFF = [0, 512, 960]
NCH = len(CHUNKS)
# instances fully covered once chunk ch's pn is written
SIMS_AT = [(0, 5), (5, 10), (10, 13)]
NJ, NG = 3, 5                # sim packing: partition blocks x col groups
NWARM = 10                   # PE warm-up matmuls during the input DMA


def _build_nc():
    nc = bacc.Bacc("TRN2", target_bir_lowering=False)
    gts = [nc.declare_dram_parameter(f"gt{ch}", [128, 2, CHUNKS[ch]], BF16,
                                     isOutput=False)
           for ch in range(NCH)]
    wcm = nc.declare_dram_parameter("wcm", [128, 2, 2, C], BF16,
                                    isOutput=False)
    bcm = nc.declare_dram_parameter("bcm", [128, 8], F32, isOutput=False)
    # per-(anchor, pos) loss terms ln(1 + S_k exp(-ap)); host reduces
    loss = nc.declare_dram_parameter("loss", [96, NG * 32], F32,
                                     isOutput=True)

    AT = mybir.ActivationFunctionType
    ALU = mybir.AluOpType
    PSUM = bass.MemorySpace.PSUM

    with tile.TileContext(nc) as tc:
        with tc.tile_pool(name="singles", bufs=1) as singles:
            # weights + gather chunks on the sync ring (weights first:
            # they gate the first LDWEIGHTS); tiny biases on the ACT ring
            WC = singles.tile([128, 2, 2, C], BF16)
            nc.sync.dma_start(out=WC[:], in_=wcm[:, :, :, :])
            gch = [singles.tile([128, 2, CHUNKS[ch]], BF16, name=f"g{ch}")
                   for ch in range(NCH)]
            for ch in range(NCH):
                nc.sync.dma_start(out=gch[ch][:], in_=gts[ch][:, :, :])
            BC = singles.tile([128, 8], F32)
            nc.scalar.dma_start(out=BC[:], in_=bcm[:, :])
            # ACT table set (covers exp/ln/square/relu/copy), after the
            # ACT-ring DMA issues but well before the first activation
            nc.scalar.add_instruction(
                mybir.InstLoadActFuncSet(
                    name=nc.get_next_instruction_name(),
                    ins=[],
                    outs=[],
                    act_func_set_id=6,  # natural_log_exp_and_others
                )
            )
            W1, W2 = WC[:, 0], WC[:, 1]
            B1, B2, blk4 = BC[:, 0:2], BC[:, 2:4], BC[:, 4:8]

            onesrf = singles.tile([1, 128], F32)
            nc.vector.memset(onesrf[:], 1.0)
            onescf = singles.tile([128, 1], F32)
            nc.vector.memset(onescf[:], 1.0)
            onesr = singles.tile([1, 128], F32R)
            nc.scalar.copy(out=onesr[:], in_=onesrf[:])
            onesc = singles.tile([128, 1], F32R)
            nc.scalar.copy(out=onesc[:], in_=onescf[:])

            with tc.tile_pool(name="big", bufs=1) as big:
                hs = [big.tile([128, STOT], BF16, name=f"hs{m}")
                      for m in range(2)]
                pn = [big.tile([128, STOT], BF16, name=f"pn{m}")
                      for m in range(2)]

                with (
                    tc.tile_pool(name="mmp", bufs=6, space=PSUM) as mmp,
                    tc.tile_pool(name="nsqp", bufs=1, space=PSUM) as nsqp,
                    tc.tile_pool(name="simp", bufs=1, space=PSUM) as simp,
                    tc.tile_pool(name="qsp", bufs=4) as qsp,
                ):
                    # one bank: sims in cols 0-319, final loss in 320-322
                    sim = simp.tile([96, 512], F32, tag="sim")

                    # PE warm-up during the input-DMA window: ramps HAM to
                    # K=8/8 before the real matmuls arrive.  Writes land in
                    # the sim bank and are overwritten later (start=True).
                    warm = singles.tile([128, 416], BF16, name="warm")
                    nc.vector.memset(warm[:], 1.0)
                    for _ in range(NWARM):
                        nc.tensor.matmul(
                            sim[:96, :320], warm[:, :96], warm[:, :320],
                            start=True, stop=True,
                        )

                    hp = {}
                    pp = {}
                    qs = {}
                    lnt = {}
                    rre = {}

                    def l1(ch):
                        cw = CHUNKS[ch]
                        hp[ch] = [mmp.tile([128, 512], F32, tag="mm",
                                           name=f"hp{ch}_{m}")
                                  for m in range(2)]
                        for k in range(2):
                            for m in range(2):
                                nc.tensor.matmul(
                                    hp[ch][m][:, :cw],
                                    W1[:, k, 128 * m : 128 * (m + 1)],
                                    gch[ch][:, k, :],
                                    start=(k == 0),
                                    stop=(k == 1),
                                )

                    def relu(ch):
                        sl = slice(COFF[ch], COFF[ch] + CHUNKS[ch])
                        for m in range(2):
                            nc.vector.tensor_scalar(
                                out=hs[m][:, sl], in0=hp[ch][m][:, :CHUNKS[ch]],
                                scalar1=B1[:, m : m + 1], scalar2=0.0,
                                op0=ALU.add, op1=ALU.max,
                            )

                    def l2(ch):
                        sl = slice(COFF[ch], COFF[ch] + CHUNKS[ch])
                        pp[ch] = [mmp.tile([128, 512], F32, tag="mm",
                                           name=f"pp{ch}_{m}")
                                  for m in range(2)]
                        for k in range(2):
                            for m in range(2):
                                nc.tensor.matmul(
                                    pp[ch][m][:, :CHUNKS[ch]],
                                    W2[:, k, 128 * m : 128 * (m + 1)],
                                    hs[k][:, sl],
                                    start=(k == 0),
                                    stop=(k == 1),
                                )

                    def sq(ch):
                        qs[ch] = []
                        for m in range(2):
                            q = qsp.tile([128, 512], F32R, tag="qs",
                                         name=f"qs{ch}_{m}")
                            nc.scalar.activation(
                                out=q[:, :CHUNKS[ch]],
                                in_=pp[ch][m][:, :CHUNKS[ch]],
                                func=AT.Square,
                                bias=B2[:, m : m + 1],
                            )
                            qs[ch].append(q)

                    nsqs = {}

                    def colsum(ch):
                        # PE: nsq = ones^T (qs0 | qs1)
                        cw = CHUNKS[ch]
                        nsq = nsqp.tile([1, 512], F32, tag="nsq")
                        for m in range(2):
                            nc.tensor.matmul(
                                nsq[:, :cw], onesc[:], qs[ch][m][:, :cw],
                                start=(m == 0), stop=(m == 1),
                            )
                        nsqs[ch] = nsq

                    def ln_op(ch):
                        cw = CHUNKS[ch]
                        t = big.tile([1, 512], F32R, tag="lnt",
                                     name="lnt", bufs=2)
                        nc.scalar.activation(
                            out=t[:, :cw], in_=nsqs[ch][:, :cw], func=AT.Ln,
                            scale=float(TAU),
                        )
                        lnt[ch] = t

                    def rrep_mm(ch):
                        # PE: broadcast ln row to 128 partitions
                        cw = CHUNKS[ch]
                        r = mmp.tile([128, 512], F32, tag="mm",
                                     name=f"rr{ch}")
                        nc.tensor.matmul(
                            r[:, :cw], onesr[:], lnt[ch][:, :cw],
                            start=True, stop=True,
                        )
                        rre[ch] = r

                    def rn_exp(ch):
                        # ACT: rn = exp(-0.5*ln(tau*nsq)), bf16 [128,chunk]
                        cw = CHUNKS[ch]
                        e = big.tile([128, 512], BF16, tag="rre",
                                     name="rre", bufs=2)
                        nc.scalar.activation(
                            out=e[:, :cw], in_=rre[ch][:, :cw], func=AT.Exp,
                            scale=-0.5,
                        )
                        rre[ch] = e

                    def pnorm(ch):
                        # DVE: pn = (pp + b2) * rn, straight from PSUM
                        cw = CHUNKS[ch]
                        sl = slice(COFF[ch], COFF[ch] + cw)
                        for m in range(2):
                            nc.vector.scalar_tensor_tensor(
                                out=pn[m][:, sl], in0=pp[ch][m][:, :cw],
                                scalar=B2[:, m : m + 1], in1=rre[ch][:, :cw],
                                op0=ALU.add, op1=ALU.mult,
                            )

                    def sims(n0, n1):
                        for n in range(n0, n1):
                            a0 = SAMP * n
                            j, g = n % NJ, n // NJ
                            dst = sim[32 * j : 32 * (j + 1),
                                      64 * g : 64 * (g + 1)]
                            for k in range(2):
                                nc.tensor.matmul(
                                    dst,
                                    pn[k][:, a0 : a0 + 32],
                                    pn[k][:, a0 + 32 : a0 + 96],
                                    start=(k == 0),
                                    stop=(k == 1),
                                )

                    # ---- max-free InfoNCE chain, split by col groups so
                    # the first part overlaps the last chunk's norm ----
                    sim3 = sim[:, : NG * 64].rearrange(
                        "p (g m) -> p g m", g=NG
                    )
                    ee = big.tile([96, NG * 32], F32, name="ee")
                    s4 = big.tile([96, NG], F32, name="s4")
                    em = big.tile([96, NG * 32], F32, name="em")
                    tt = big.tile([96, NG * 32], F32, name="tt")
                    ctb = big.tile([96, NG * 32], F32, name="ctb")

                    def chain(g0, g1):
                        gn = g1 - g0
                        c0, c1 = 32 * g0, 32 * g1
                        ee3 = ee[:, c0:c1].rearrange("p (g m) -> p g m", g=gn)
                        em3 = em[:, c0:c1].rearrange("p (g m) -> p g m", g=gn)
                        nc.scalar.activation(
                            out=ee3, in_=sim3[:, g0:g1, 32:64], func=AT.Exp,
                        )
                        nc.vector.reduce_sum(
                            out=s4[:, g0:g1], in_=ee3,
                            axis=mybir.AxisListType.X,
                        )
                        nc.scalar.activation(
                            out=em3, in_=sim3[:, g0:g1, 0:32], func=AT.Exp,
                            scale=-1.0,
                        )
                        nc.vector.tensor_mul(
                            out=tt[:, c0:c1].rearrange(
                                "p (g m) -> p g m", g=gn
                            ),
                            in0=em3,
                            in1=s4[:, g0:g1].unsqueeze(-1).broadcast_to(
                                [96, gn, 32]
                            ),
                        )
                        nc.scalar.activation(
                            out=ctb[:, c0:c1], in_=tt[:, c0:c1], func=AT.Ln,
                            bias=1.0,
                        )

                    # ---- software-pipelined issue order ----
                    l1(0); relu(0)
                    l1(1); relu(1)
                    l1(2); relu(2)
                    l2(0); sq(0); colsum(0); ln_op(0)
                    l2(1); sq(1); rrep_mm(0); rn_exp(0); pnorm(0)
                    colsum(1)
                    l2(2); ln_op(1); sq(2)
                    rrep_mm(1); rn_exp(1); pnorm(1)
                    sims(0, 5); sims(5, 10)
                    colsum(2); ln_op(2); rrep_mm(2); rn_exp(2); pnorm(2)
                    chain(0, 3)          # instances 0-8, overlaps chunk 2
                    sims(10, NI)
                    chain(3, NG)         # instances 9-12
                    # garbage in the two unused slots (g=4, j=1,2) stays
                    # in its blocks; the host only reads valid ones
                    nc.sync.dma_start(out=loss[:, :], in_=ctb[:])

    nc.compile()
    return nc


_NC_CACHE = None


def _get_nc():
    global _NC_CACHE
    if _NC_CACHE is None:
        _NC_CACHE = _build_nc()
    return _NC_CACHE


def _host_prep(feats, w1, b1, w2, b2, anchor_inds, pos_inds, neg_inds):
    """Build the 8 per-core input maps."""
    n = feats.shape[0]
    ntot = N_CORES * NI
    ff = np.asarray(feats, dtype=np.float32).reshape(n, C, PIX)

    def flat(inds):
        inds = np.asarray(inds)
        f = inds[..., 0].astype(np.int64) * SIDE + inds[..., 1].astype(np.int64)
        if ntot > n:
            f = np.concatenate(
                [f, np.broadcast_to(f[0], (ntot - n,) + f.shape[1:])], axis=0
            )
        return f  # [ntot, 32]

    af, pf, nf = flat(anchor_inds), flat(pos_inds), flat(neg_inds)
    samp = np.concatenate([af, pf, nf], axis=1)  # [ntot, 96]
    idx = np.minimum(np.arange(ntot), n - 1)
    g = np.take_along_axis(ff[idx], samp[:, None, :], axis=2)
    # per-core channel-major [C, 1248] -> bf16 chunks [128, 2, cw]
    g = g.reshape(N_CORES, NI, C, SAMP)
    g = np.transpose(g, (0, 2, 1, 3)).reshape(N_CORES, C, STOT)
    g = g.astype(ml_dtypes.bfloat16)
    g = g.reshape(N_CORES, 2, 128, STOT)  # c = k*128 + p
    g = np.transpose(g, (0, 2, 1, 3))     # [cores, 128, 2, STOT]
    gchunks = [
        np.ascontiguousarray(g[:, :, :, COFF[ch] : COFF[ch] + CHUNKS[ch]])
        for ch in range(NCH)
    ]

    def wprep(w):
        wt = np.asarray(w, dtype=np.float32).T  # [c, d]
        wt = wt.reshape(2, 128, C)              # [k, p, d]
        return np.transpose(wt, (1, 0, 2)).astype(ml_dtypes.bfloat16)

    wcm = np.ascontiguousarray(
        np.stack([wprep(w1), wprep(w2)], axis=1)
    )  # [128, 2, 2, C]
    bcm = np.zeros((128, 8), dtype=np.float32)
    bcm[:, 0:2] = np.asarray(b1, dtype=np.float32).reshape(2, 128).T
    bcm[:, 2:4] = np.asarray(b2, dtype=np.float32).reshape(2, 128).T
    for j in range(4):
        bcm[32 * j : 32 * (j + 1), 4 + j] = 1.0

    in_maps = []
    for c in range(N_CORES):
        m = {"wcm": wcm, "bcm": bcm}
        for ch in range(NCH):
            m[f"gt{ch}"] = gchunks[ch][c]
        in_maps.append(m)
    return in_maps


def _finalize(loss_per, gt_mask):
    gt = np.asarray(gt_mask)
    area = gt.reshape(gt.shape[0], -1).sum(axis=1)
    valid = (area > NUM_SAMPLES) & (area < PIX - NUM_SAMPLES)
    n_valid = np.float32(valid.sum())
    if n_valid > 0:
        total = np.float32(np.where(valid, loss_per, 0.0).astype(np.float32).sum())
        out = total / max(n_valid, np.float32(1.0))
    else:
        out = np.float32(0.0)
    return np.float32(out * np.float32(LOSS_WEIGHT))


def kernel(feats, w1, b1, w2, b2, gt_mask, anchor_inds, pos_inds, neg_inds,
           _results_hook=None):
    nc = _get_nc()
    in_maps = _host_prep(feats, w1, b1, w2, b2, anchor_inds, pos_inds, neg_inds)
    res = run_bass_kernel_spmd(nc, in_maps, list(range(N_CORES)))
    if _results_hook is not None:
        _results_hook(res)
    parts = []
    for c in range(N_CORES):
        ctb = res.results[c]["loss"]  # [96, NG*32]
        for n in range(NI):
            j, g = n % NJ, n // NJ
            blk = ctb[32 * j : 32 * (j + 1), 32 * g : 32 * (g + 1)]
            parts.append(blk.sum(dtype=np.float32))
    loss_per = np.array(parts, dtype=np.float32)[: N_INST] / np.float32(
        NUM_SAMPLES * NUM_SAMPLES
    )
    return _finalize(loss_per, gt_mask)
